# revision 13
# baseline (speedup 1.0000x reference)
"""Trainium2 Bass kernel for nn_CausalityChainModel (loss_fn), 8-core SPMD.

Self-contained: takes FULL inputs, shards internally across 8 NeuronCores,
runs one Bass/Tile program via run_bass_kernel_spmd, returns the scalar loss.

Key math (validated numerically against the reference on CPU):
- loss_indep's [n,N,n] residual tensor collapses analytically:
      G[j,i,k] = S[i,k] - S[j,i]S[j,k]/s2[j]
  (S = centered Gram of X_ind), and the masked weighted triple sum reduces to
  a handful of [64,64] matrix products.
- BatchNorm (train-mode, biased var) stats come from raw Gram matrices of the
  layer inputs: E[h] = W1 colsum(x)/N, E[h^2] = diag(W1 G W1^T)/N, G = x^T x.
  So BN+LeakyReLU is one ACT pass: Lrelu(psum*scale + bias).
- Large matmuls in bf16; the X_ind chain in float32r (full-rate, ~1e-3 rel);
  the Grams feeding X_ind-path BN stats in full fp32.
- Sharding: sample axes of z_logits/X/noise_indep split across cores;
  noise_trans (Zp) replicated; NCT candidates (Zs) sharded over j with a
  min-combine in the final AllGather.
- Collectives (AllGather only, queued in order): AG1 gram-z partials (first,
  absorbs the ~70us ncfw cold-start under local compute), AG2 X_ind-h BN stat
  sums, AG3 S-gram/colsum/mse partials + per-i distance mins.
"""
import os
import sys
import types
import contextlib

for _p in ("/opt/trn_rl_repo", "/root/.axon_site"):
    if _p not in sys.path:
        sys.path.insert(0, _p)

import numpy as np
import ml_dtypes

import concourse.bass as bass
import concourse.tile as tile
from concourse import mybir
from concourse.bass_utils import run_bass_kernel_spmd

SIZE, NS, LAT, NOISE, HID, BTR, NIND = 64, 16384, 128, 64, 256, 2048, 8192
NCORES = 8
SH_NS = NS // NCORES      # 2048
SH_NI = NIND // NCORES    # 1024
SH_J = NS // NCORES       # 2048 Zs rows per core
BN_EPS = 1e-5
LRELU = 0.01

f32 = mybir.dt.float32
f32r = mybir.dt.float32r
bf16 = mybir.dt.bfloat16
i32 = mybir.dt.int32
AF = mybir.ActivationFunctionType
ALU = mybir.AluOpType
AX = mybir.AxisListType
bfnp = ml_dtypes.bfloat16

AG1F = LAT + 1            # gram-z partial columns
AG2F = 8                  # sum(h2) x4 chunks, sum(h2^2) x4 chunks
AG3F = 98                 # 0-63 S, 64 colsum, 65 mse, 66-97 dmin (32 cols)
NADD = 66
NI_CH = 16
BIGF = 3.0e38

_CACHE = {}


def _install_profshim():
    if "antenv.axon_hooks" in sys.modules:
        return
    try:
        import antenv
        mod = types.ModuleType("antenv.axon_hooks")
        mod._hook = None
        mod.set_axon_ntff_profile_hook = lambda h: setattr(mod, "_hook", h)
        mod.get_axon_ntff_profile_hook = lambda: mod._hook
        sys.modules["antenv.axon_hooks"] = mod
        antenv.axon_hooks = mod
        from trn_agent_boot import trn_boot
        so = "/opt/axon/libaxon_pjrt.so"
        if os.path.exists(so):
            mod.set_axon_ntff_profile_hook(trn_boot._ntff_profile_via_ctypes(so))
        import concourse.bass_utils as bu
        bu.upload_artifacts = lambda tmpdir: str(tmpdir)
    except Exception:
        pass


def _split_multi_waits(nc, max_waits=1):
    """This walrus build rejects >1 sem-wait per instruction: move extras onto
    EventSemaphore nops (cheap, non-pipeline-flushing) placed just before."""
    for bb in nc.main_func.blocks:
        new_insts = []
        for inst in bb.instructions:
            si = inst.sync_info
            if si is not None and len(si.on_wait) > max_waits:
                waits = list(si.on_wait)
                extra, keep = waits[:-max_waits], waits[-max_waits:]
                for i in range(0, len(extra), max_waits):
                    d = mybir.InstEventSemaphore(
                        name=f"{inst.name}-wsplit{i}", ins=[], outs=[])
                    d.engine = inst.engine
                    d.sync_info = mybir.SyncInfo(
                        on_wait=list(extra[i:i + max_waits]), on_update=[])
                    new_insts.append(d)
                inst.sync_info = mybir.SyncInfo(
                    on_wait=list(keep), on_update=list(si.on_update))
            new_insts.append(inst)
        try:
            bb.instructions[:] = new_insts
        except TypeError:
            bb.instructions = new_insts


def _build_program():
    nc = bass.Bass()

    def din(name, shape, dt):
        return nc.dram_tensor(name, shape, dt, kind="ExternalInput")

    zext = din("zext", [NS, LAT + 1], bf16)            # gather source
    znat32 = din("znat32", [SH_NS, LAT + 1], f32)      # shard, z|ones fp32
    zT_sh = din("zT_sh", [LAT, SH_NS], bf16)
    xT_sh = din("xT_sh", [SIZE, SH_NS], bf16)
    ntrT = din("ntrT", [NOISE, BTR], bf16)
    ntr_ext = din("ntr_ext", [BTR, NOISE + 1], bf16)
    nind_e32 = din("nind_e32", [NIND, NOISE + 1], f32)
    nindT32 = din("nindT32", [NOISE, SH_NI], f32r)
    perm_sh = din("perm_sh", [SH_J, 1], i32)
    gW1T_bf_d = din("gW1T_bf", [LAT, HID], bf16)
    gW1nat_d = din("gW1nat_bf", [HID, LAT], bf16)
    gW2T_bf_d = din("gW2T_bf", [HID, SIZE], bf16)
    gW2T_32_d = din("gW2T_32", [HID, SIZE], f32r)
    gW1T_32_d = din("gW1T_32", [LAT, HID], f32r)
    tW1T_bf_d = din("tW1T_bf", [NOISE, HID], bf16)
    tW1nat_d = din("tW1nat_bf", [HID, NOISE], bf16)
    tW1T_32_d = din("tW1T_32", [NOISE, HID], f32r)
    tW2T_bf_d = din("tW2T_bf", [HID, LAT], bf16)
    tW2T_32_d = din("tW2T_32", [HID, LAT], f32r)
    g_gam_d = din("g_gam", [HID, 1], f32)
    g_bet_d = din("g_bet", [HID, 1], f32)
    t_gam_d = din("t_gam", [HID, 1], f32)
    t_bet_d = din("t_bet", [HID, 1], f32)
    g_b2_d = din("g_b2", [SIZE, 1], f32)
    t_b2_d = din("t_b2", [LAT, 1], f32)
    L32_d = din("L32", [SIZE, SIZE], f32)
    LT32_d = din("LT32", [SIZE, SIZE], f32)
    eye64_d = din("eye64", [SIZE, SIZE], f32)
    offd64_d = din("offd64", [SIZE, SIZE], f32)
    ident_bf_d = din("ident_bf", [128, 128], bf16)
    ident_32_d = din("ident_32", [128, 128], f32)
    identr_d = din("identr", [128, 128], f32r)
    ones_row_d = din("ones_row_bf", [1, 128], bf16)
    ones_col_d = din("ones_col_bf", [128, 1], bf16)
    ones64_d = din("ones64_32", [SIZE, 1], f32)
    ones128_d = din("ones128_32", [128, 1], f32)

    out_d = nc.dram_tensor("out", [1, 1], f32, kind="ExternalOutput")

    ag1_out = nc.dram_tensor("ag1_out", [NCORES * 128, AG1F], f32,
                             addr_space="Shared")
    ag2_out = nc.dram_tensor("ag2_out", [NCORES * 128, AG2F], f32,
                             addr_space="Shared")
    ag3_out = nc.dram_tensor("ag3_out", [NCORES * 128, AG3F], f32,
                             addr_space="Shared")

    with tile.TileContext(nc) as tc, contextlib.ExitStack() as ctx:
        const = ctx.enter_context(tc.tile_pool(name="const", bufs=1))
        sb = ctx.enter_context(tc.tile_pool(name="sb", bufs=1))
        sb3 = ctx.enter_context(tc.tile_pool(name="sb3", bufs=4))
        ps_acc = ctx.enter_context(tc.tile_pool(name="ps_acc", bufs=2, space="PSUM"))
        ps_sm = ctx.enter_context(tc.tile_pool(name="ps_sm", bufs=2, space="PSUM"))
        ps_d = ctx.enter_context(tc.tile_pool(name="ps_d", bufs=2, space="PSUM"))
        dram = ctx.enter_context(tc.tile_pool(name="dram", bufs=1, space="DRAM"))

        # ---------------- input loads
        def load(shape, dt, src, name):
            t = const.tile(shape, dt, tag=name, name=name)
            nc.sync.dma_start(out=t[:], in_=src)
            return t

        def load2(shape, dt, src_d, name):
            return [load([128, shape[1]], dt,
                         src_d[b * 128:(b + 1) * 128, :], f"{name}{b}")
                    for b in range(2)]

        t_znat = sb.tile([128, SH_NS // 128, LAT + 1], f32, name="t_znat")
        nc.sync.dma_start(out=t_znat[:],
                          in_=znat32[:].rearrange("(c p) f -> p c f", p=128))

        ident_bf = load([128, 128], bf16, ident_bf_d[:], "ident_bf")
        ident_32 = load([128, 128], f32, ident_32_d[:], "ident_32")
        identr = load([128, 128], f32r, identr_d[:], "identr")
        ones_row = load([1, 128], bf16, ones_row_d[:], "ones_row")
        ones_col = load([128, 1], bf16, ones_col_d[:], "ones_col")
        ones64 = load([SIZE, 1], f32, ones64_d[:], "ones64")
        ones128 = load([128, 1], f32, ones128_d[:], "ones128")
        eps_col = const.tile([128, 1], f32, tag="eps_col", name="eps_col")
        nc.vector.memset(eps_col[:], BN_EPS)
        eye = load([SIZE, SIZE], f32, eye64_d[:], "eye")
        offd = load([SIZE, SIZE], f32, offd64_d[:], "offd")
        Lc = load([SIZE, SIZE], f32, L32_d[:], "L")
        LTc = load([SIZE, SIZE], f32, LT32_d[:], "LT")
        gW1T_bf = load([LAT, HID], bf16, gW1T_bf_d[:], "gW1T_bf")
        gW1nat = load2([HID, LAT], bf16, gW1nat_d, "gW1nat")
        gW2T_bf = load2([HID, SIZE], bf16, gW2T_bf_d, "gW2T_bf")
        gW2T_32 = load2([HID, SIZE], f32r, gW2T_32_d, "gW2T_32")
        gW1T_32 = load([LAT, HID], f32r, gW1T_32_d[:], "gW1T_32")
        tW1T_bf = load([NOISE, HID], bf16, tW1T_bf_d[:], "tW1T_bf")
        tW1nat = load2([HID, NOISE], bf16, tW1nat_d, "tW1nat")
        tW1T_32 = load([NOISE, HID], f32r, tW1T_32_d[:], "tW1T_32")
        tW2T_bf = load2([HID, LAT], bf16, tW2T_bf_d, "tW2T_bf")
        tW2T_32 = load2([HID, LAT], f32r, tW2T_32_d, "tW2T_32")
        g_b2 = load([SIZE, 1], f32, g_b2_d[:], "g_b2")
        t_b2 = load([LAT, 1], f32, t_b2_d[:], "t_b2")
        g_gam = [load([128, 1], f32, g_gam_d[b * 128:(b + 1) * 128, :], f"g_gam{b}")
                 for b in range(2)]
        g_bet = [load([128, 1], f32, g_bet_d[b * 128:(b + 1) * 128, :], f"g_bet{b}")
                 for b in range(2)]
        t_gam = [load([128, 1], f32, t_gam_d[b * 128:(b + 1) * 128, :], f"t_gam{b}")
                 for b in range(2)]
        t_bet = [load([128, 1], f32, t_bet_d[b * 128:(b + 1) * 128, :], f"t_bet{b}")
                 for b in range(2)]

        t_zT = sb.tile([LAT, SH_NS], bf16, name="t_zT")
        nc.sync.dma_start(out=t_zT[:], in_=zT_sh[:])
        t_xT = sb.tile([SIZE, SH_NS], bf16, name="t_xT")
        nc.sync.dma_start(out=t_xT[:], in_=xT_sh[:])
        t_ntrT = sb.tile([NOISE, BTR], bf16, name="t_ntrT")
        nc.sync.dma_start(out=t_ntrT[:], in_=ntrT[:])
        t_nindT = sb.tile([NOISE, SH_NI], f32r, name="t_nindT")
        nc.sync.dma_start(out=t_nindT[:], in_=nindT32[:])
        t_perm = sb.tile([128, NI_CH], i32, name="t_perm")
        nc.sync.dma_start(out=t_perm[:],
                          in_=perm_sh[:].rearrange("(g p) o -> p (g o)", p=128))

        # ---------------- AG1: sharded fp32 gram of z (16 fp32 matmuls)
        gz_ps = ps_acc.tile([LAT, AG1F], f32, tag="acc", name="gz_ps")
        for k in range(SH_NS // 128):
            nc.tensor.matmul(out=gz_ps[:], lhsT=t_znat[:, k, :LAT],
                             rhs=t_znat[:, k, :],
                             start=(k == 0), stop=(k == SH_NS // 128 - 1))
        pay1 = sb.tile([128, AG1F], f32, name="pay1")
        nc.scalar.copy(out=pay1[:], in_=gz_ps[:])
        ag1_in = dram.tile([128, AG1F], f32, name="ag1_in")
        nc.sync.dma_start(out=ag1_in[:], in_=pay1[:])
        nc.gpsimd.collective_compute(
            "AllGather", ALU.bypass, ins=[ag1_in[:].opt()],
            outs=[ag1_out[:].opt()], replica_groups=[list(range(NCORES))])

        # ---------------- replicated grams: noise_trans (bf16), noise_ind (fp32)
        def gram_from_dram(src, dtype, nrows, nin, tag):
            nch = nrows // 128
            grp = 8
            gps = ps_acc.tile([nin, nin + 1], f32, tag="acc", name=f"g_{tag}")
            view = src[:].rearrange("(c p) f -> p c f", p=128)
            for k0 in range(0, nch, grp):
                stage = sb3.tile([128, grp, nin + 1], dtype, tag=f"gs_{tag}",
                                 name=f"gs_{tag}")
                nc.sync.dma_start(out=stage[:], in_=view[:, k0:k0 + grp, :])
                for j in range(grp):
                    k = k0 + j
                    nc.tensor.matmul(out=gps[:], lhsT=stage[:, j, :nin],
                                     rhs=stage[:, j, :],
                                     start=(k == 0), stop=(k == nch - 1))
            gsb = sb.tile([nin, nin + 1], f32, tag=f"gss_{tag}", name=f"gss_{tag}")
            nc.scalar.copy(out=gsb[:], in_=gps[:])
            return gsb

        gtr = gram_from_dram(ntr_ext, bf16, BTR, NOISE, "tr")
        gni = gram_from_dram(nind_e32, f32, NIND, NOISE, "ni")

        # ---------------- BN stats from a Gram
        def _stat_tail(esq_or_tot2, mu, gam, bet, N, tag):
            var = sb.tile([128, 1], f32, tag=f"var_{tag}", name=f"var_{tag}")
            nc.scalar.activation(out=var[:], in_=esq_or_tot2[:], func=AF.Copy,
                                 scale=1.0 / N)
            musq = sb.tile([128, 1], f32, tag="stat_musq", name="stat_musq")
            nc.vector.tensor_tensor(out=musq[:], in0=mu[:], in1=mu[:], op=ALU.mult)
            nc.vector.tensor_tensor(out=var[:], in0=var[:], in1=musq[:],
                                    op=ALU.subtract)
            std = sb.tile([128, 1], f32, tag="stat_std", name="stat_std")
            nc.scalar.activation(out=std[:], in_=var[:], func=AF.Sqrt,
                                 bias=eps_col[:])
            rstd = sb.tile([128, 1], f32, tag="stat_rstd", name="stat_rstd")
            nc.vector.reciprocal(out=rstd[:], in_=std[:])
            s = sb.tile([128, 1], f32, tag=f"s_{tag}", name=f"s_{tag}")
            nc.vector.tensor_tensor(out=s[:], in0=gam[:], in1=rstd[:], op=ALU.mult)
            bb_ = sb.tile([128, 1], f32, tag=f"b_{tag}", name=f"b_{tag}")
            nc.vector.tensor_tensor(out=bb_[:], in0=mu[:], in1=s[:], op=ALU.mult)
            nc.vector.tensor_tensor(out=bb_[:], in0=bet[:], in1=bb_[:],
                                    op=ALU.subtract)
            return s, bb_

        def stats_from_gram(gram, w1T, w1nat, gam, bet, n_in, N, tag,
                            use_bf=True):
            if use_bf:
                gmm = sb.tile([n_in, n_in + 1], bf16, tag=f"gb_{tag}",
                              name=f"gb_{tag}")
                nc.scalar.copy(out=gmm[:], in_=gram[:])
            else:
                gmm = gram
            scales, biases = [], []
            for b in range(2):
                mm = ps_acc.tile([128, n_in + 1], f32, tag="acc", name="stat_mm")
                nc.tensor.matmul(out=mm[:], lhsT=w1T[:, b * 128:(b + 1) * 128],
                                 rhs=gmm[:], start=True, stop=True)
                prod = sb.tile([128, n_in], f32, tag="stat_prod", name="stat_prod")
                nc.vector.tensor_tensor(out=prod[:], in0=mm[:, :n_in],
                                        in1=w1nat[b][:], op=ALU.mult)
                esq = sb.tile([128, 1], f32, tag=f"esq_{tag}{b}",
                              name=f"esq_{tag}{b}")
                nc.vector.reduce_sum(out=esq[:], in_=prod[:], axis=AX.X)
                mu = sb.tile([128, 1], f32, tag=f"mu_{tag}{b}", name=f"mu_{tag}{b}")
                nc.scalar.activation(out=mu[:], in_=mm[:, n_in:n_in + 1],
                                     func=AF.Copy, scale=1.0 / N)
                s, bias = _stat_tail(esq, mu, gam[b], bet[b], N, f"{tag}{b}")
                scales.append(s)
                biases.append(bias)
            return scales, biases

        tr_s, tr_b = stats_from_gram(gtr, tW1T_bf, tW1nat, t_gam, t_bet,
                                     NOISE, BTR, "tr")
        # stats matmuls for the ind path also in bf16 weights but fp32 gram:
        # mixed dtypes are not allowed -> cast gram to bf16 would lose the
        # fp32 gain; instead run these two stat matmuls in fp32.
        ind_s, ind_b = [], []
        for b in range(2):
            mm = ps_acc.tile([128, NOISE + 1], f32, tag="acc", name="istat_mm")
            # fp32 matmul: lhsT fp32 [64, 128], rhs fp32 [64, 65]
            tW1T_f = sb.tile([NOISE, 128], f32, tag=f"tW1Tf{b}", name=f"tW1Tf{b}")
            nc.vector.tensor_copy(out=tW1T_f[:], in_=tW1T_32[:, b * 128:(b + 1) * 128])
            nc.tensor.matmul(out=mm[:], lhsT=tW1T_f[:], rhs=gni[:],
                             start=True, stop=True)
            prod = sb.tile([128, NOISE], f32, tag="stat_prod", name="stat_prod")
            nc.vector.tensor_tensor(out=prod[:], in0=mm[:, :NOISE],
                                    in1=tW1nat[b][:], op=ALU.mult)
            esq = sb.tile([128, 1], f32, tag=f"esq_ind{b}", name=f"esq_ind{b}")
            nc.vector.reduce_sum(out=esq[:], in_=prod[:], axis=AX.X)
            mu = sb.tile([128, 1], f32, tag=f"mu_ind{b}", name=f"mu_ind{b}")
            nc.scalar.activation(out=mu[:], in_=mm[:, NOISE:NOISE + 1],
                                 func=AF.Copy, scale=1.0 / NIND)
            s, bias = _stat_tail(esq, mu, t_gam[b], t_bet[b], NIND, f"ind{b}")
            ind_s.append(s)
            ind_b.append(bias)

        # ---------------- tr branch: Zp (replicated), -2*(Zp+b2)
        h_tr = [sb.tile([128, BTR], bf16, tag=f"h_tr{b}", name=f"h_tr{b}")
                for b in range(2)]
        for b in range(2):
            for n in range(BTR // 512):
                hp = ps_sm.tile([128, 512], f32, tag="sm", name="hmm")
                nc.tensor.matmul(out=hp[:], lhsT=tW1T_bf[:, b * 128:(b + 1) * 128],
                                 rhs=t_ntrT[:, n * 512:(n + 1) * 512],
                                 start=True, stop=True)
                nc.scalar.activation(out=h_tr[b][:, n * 512:(n + 1) * 512],
                                     in_=hp[:], func=AF.Lrelu,
                                     bias=tr_b[b][:], scale=tr_s[b][:],
                                     alpha=LRELU)
        zpm2 = sb.tile([LAT, BTR], bf16, name="zpm2")
        for n in range(BTR // 512):
            zp = ps_sm.tile([LAT, 512], f32, tag="sm", name="zpmm")
            for b in range(2):
                nc.tensor.matmul(out=zp[:], lhsT=tW2T_bf[b][:],
                                 rhs=h_tr[b][:, n * 512:(n + 1) * 512],
                                 start=(b == 0), stop=(b == 1))
            nc.vector.tensor_scalar(out=zpm2[:, n * 512:(n + 1) * 512], in0=zp[:],
                                    scalar1=t_b2[:], scalar2=-2.0,
                                    op0=ALU.add, op1=ALU.mult)
        zpsq_scr = sb.tile([LAT, BTR], bf16, tag="sq128", name="zpsq_scr")
        zpsq_col = sb.tile([128, 1], f32, name="zpsq_col")
        nc.scalar.activation(out=zpsq_scr[:], in_=zpm2[:], func=AF.Square,
                             accum_out=zpsq_col[:])

        # ---------------- Zs gather + transpose + nsq broadcast rows
        zsT = sb.tile([LAT, SH_J], bf16, name="zsT")
        for g in range(NI_CH):
            gz_t = sb3.tile([128, LAT + 1], bf16, tag="zs_gather", name="zs_gather")
            nc.gpsimd.indirect_dma_start(
                out=gz_t[:], out_offset=None, in_=zext[:],
                in_offset=bass.IndirectOffsetOnAxis(ap=t_perm[:, g:g + 1], axis=0))
            tp = ps_sm.tile([128, 128], bf16, tag="sm", name="zs_tp")
            nc.tensor.transpose(out=tp[:], in_=gz_t[:, :LAT], identity=ident_bf[:])
            nc.scalar.copy(out=zsT[:, g * 128:(g + 1) * 128], in_=tp[:])
        zsq = sb.tile([LAT, SH_J], bf16, tag="sq128", name="zsq")
        nc.scalar.activation(out=zsq[:], in_=zsT[:], func=AF.Square)
        nsq_row = sb.tile([1, SH_J], bf16, name="nsq_row")
        for n in range(SH_J // 512):
            np_ = ps_sm.tile([1, 512], f32, tag="sm", name="nsqp")
            nc.tensor.matmul(out=np_[:], lhsT=ones_col[:],
                             rhs=zsq[:, n * 512:(n + 1) * 512],
                             start=True, stop=True)
            nc.scalar.copy(out=nsq_row[:, n * 512:(n + 1) * 512], in_=np_[:])

        # ---------------- ind chain (f32r): h_ind -> Z_ind -> h2 (+ stat sums)
        h_ind = [sb.tile([128, SH_NI], f32r, tag=f"h_ind{b}", name=f"h_ind{b}")
                 for b in range(2)]
        for b in range(2):
            for n in range(SH_NI // 512):
                hp = ps_sm.tile([128, 512], f32, tag="sm", name="himm")
                nc.tensor.matmul(out=hp[:], lhsT=tW1T_32[:, b * 128:(b + 1) * 128],
                                 rhs=t_nindT[:, n * 512:(n + 1) * 512],
                                 start=True, stop=True)
                nc.scalar.activation(out=h_ind[b][:, n * 512:(n + 1) * 512],
                                     in_=hp[:], func=AF.Lrelu,
                                     bias=ind_b[b][:], scale=ind_s[b][:],
                                     alpha=LRELU)
        ziT = sb.tile([LAT, SH_NI], f32r, name="ziT")
        for n in range(SH_NI // 512):
            zp = ps_sm.tile([LAT, 512], f32, tag="sm", name="zimm")
            for b in range(2):
                nc.tensor.matmul(out=zp[:], lhsT=tW2T_32[b][:],
                                 rhs=h_ind[b][:, n * 512:(n + 1) * 512],
                                 start=(b == 0), stop=(b == 1))
            nc.vector.tensor_scalar_add(out=ziT[:, n * 512:(n + 1) * 512],
                                        in0=zp[:], scalar1=t_b2[:])
        pay2 = sb.tile([128, AG2F], f32, name="pay2")
        h2 = [sb.tile([128, SH_NI], f32r, tag=f"h2_{b}", name=f"h2_{b}")
              for b in range(2)]
        sq_scr = sb.tile([128, 512], f32, tag="sqscr32", name="sq_scr")
        for b in range(2):
            for n in range(SH_NI // 512):
                hp = ps_sm.tile([128, 512], f32, tag="sm", name="h2mm")
                nc.tensor.matmul(out=hp[:], lhsT=gW1T_32[:, b * 128:(b + 1) * 128],
                                 rhs=ziT[:, n * 512:(n + 1) * 512],
                                 start=True, stop=True)
                col = b * 2 + n
                nc.scalar.activation(out=h2[b][:, n * 512:(n + 1) * 512],
                                     in_=hp[:], func=AF.Copy,
                                     accum_out=pay2[:, col:col + 1])
                nc.scalar.activation(out=sq_scr[:],
                                     in_=h2[b][:, n * 512:(n + 1) * 512],
                                     func=AF.Square,
                                     accum_out=pay2[:, 4 + col:5 + col])
        ag2_in = dram.tile([128, AG2F], f32, name="ag2_in")
        nc.sync.dma_start(out=ag2_in[:], in_=pay2[:])
        nc.gpsimd.collective_compute(
            "AllGather", ALU.bypass, ins=[ag2_in[:].opt()],
            outs=[ag2_out[:].opt()], replica_groups=[list(range(NCORES))])

        # ---------------- NCT distance loop (overlaps AG1/AG2)
        pay3 = sb.tile([128, AG3F], f32, name="pay3")
        nc.vector.memset(pay3[:], 0.0)
        for ic in range(NI_CH):
            for jh in range(2):
                dps = ps_d.tile([128, 1024], f32, tag="dps", name="dps")
                for jq in range(2):
                    off = jh * 1024 + jq * 512
                    sl = slice(jq * 512, (jq + 1) * 512)
                    nc.tensor.matmul(out=dps[:, sl], lhsT=ones_row[:],
                                     rhs=nsq_row[:, off:off + 512],
                                     start=True, stop=False)
                    nc.tensor.matmul(out=dps[:, sl],
                                     lhsT=zpm2[:, ic * 128:(ic + 1) * 128],
                                     rhs=zsT[:, off:off + 512],
                                     start=False, stop=True)
                col = NADD + ic * 2 + jh
                nc.vector.tensor_reduce(out=pay3[:, col:col + 1], in_=dps[:],
                                        axis=AX.X, op=ALU.min)

        # ---------------- AG1 combine -> glo stats -> glo branch -> mse
        ag1l = sb.tile([128, AG1F, NCORES], f32, name="ag1l")
        nc.sync.dma_start(out=ag1l[:],
                          in_=ag1_out[:].rearrange("(c p) f -> p f c", p=128))
        gz = sb.tile([128, AG1F], f32, name="gz")
        nc.vector.reduce_sum(out=gz[:], in_=ag1l[:], axis=AX.X)
        glo_s, glo_b = stats_from_gram(gz, gW1T_bf, gW1nat, g_gam, g_bet,
                                       LAT, NS, "glo")
        h_glo = [sb.tile([128, SH_NS], bf16, tag=f"h_glo{b}", name=f"h_glo{b}")
                 for b in range(2)]
        for b in range(2):
            for n in range(SH_NS // 512):
                hp = ps_sm.tile([128, 512], f32, tag="sm", name="hgmm")
                nc.tensor.matmul(out=hp[:], lhsT=gW1T_bf[:, b * 128:(b + 1) * 128],
                                 rhs=t_zT[:, n * 512:(n + 1) * 512],
                                 start=True, stop=True)
                nc.scalar.activation(out=h_glo[b][:, n * 512:(n + 1) * 512],
                                     in_=hp[:], func=AF.Lrelu,
                                     bias=glo_b[b][:], scale=glo_s[b][:],
                                     alpha=LRELU)
        dtile = sb.tile([SIZE, SH_NS], f32, name="dtile")
        for n in range(SH_NS // 512):
            xp = ps_sm.tile([SIZE, 512], f32, tag="sm", name="xgmm")
            for b in range(2):
                nc.tensor.matmul(out=xp[:], lhsT=gW2T_bf[b][:],
                                 rhs=h_glo[b][:, n * 512:(n + 1) * 512],
                                 start=(b == 0), stop=(b == 1))
            nc.vector.scalar_tensor_tensor(
                out=dtile[:, n * 512:(n + 1) * 512], in0=xp[:], scalar=g_b2[:],
                in1=t_xT[:, n * 512:(n + 1) * 512], op0=ALU.add, op1=ALU.subtract)
        msesq = sb.tile([SIZE, SH_NS], bf16, tag="sq64", name="msesq")
        nc.scalar.activation(out=msesq[:], in_=dtile[:], func=AF.Square,
                             accum_out=pay3[:SIZE, 65:66])

        # ---------------- AG2 combine -> X_ind -> S partials
        ag2l = sb.tile([128, AG2F, NCORES], f32, name="ag2l")
        nc.sync.dma_start(out=ag2l[:],
                          in_=ag2_out[:].rearrange("(c p) f -> p f c", p=128))
        sums2 = sb.tile([128, AG2F], f32, name="sums2")
        nc.vector.reduce_sum(out=sums2[:], in_=ag2l[:], axis=AX.X)
        h2_s, h2_b = [], []
        for b in range(2):
            tot = sb.tile([128, 1], f32, tag=f"h2tot{b}", name=f"h2tot{b}")
            nc.vector.tensor_tensor(out=tot[:], in0=sums2[:, 2 * b:2 * b + 1],
                                    in1=sums2[:, 2 * b + 1:2 * b + 2], op=ALU.add)
            mu = sb.tile([128, 1], f32, tag=f"h2mu{b}", name=f"h2mu{b}")
            nc.scalar.activation(out=mu[:], in_=tot[:], func=AF.Copy,
                                 scale=1.0 / NIND)
            tot2 = sb.tile([128, 1], f32, tag=f"h2tot2{b}", name=f"h2tot2{b}")
            nc.vector.tensor_tensor(out=tot2[:], in0=sums2[:, 4 + 2 * b:5 + 2 * b],
                                    in1=sums2[:, 5 + 2 * b:6 + 2 * b], op=ALU.add)
            s, bb_ = _stat_tail(tot2, mu, g_gam[b], g_bet[b], NIND, f"h2{b}")
            h2_s.append(s)
            h2_b.append(bb_)
        h2a = [sb.tile([128, SH_NI], f32r, tag=f"h2a{b}", name=f"h2a{b}")
               for b in range(2)]
        for b in range(2):
            nc.scalar.activation(out=h2a[b][:], in_=h2[b][:], func=AF.Lrelu,
                                 bias=h2_b[b][:], scale=h2_s[b][:], alpha=LRELU)
        xiT = sb.tile([SIZE, SH_NI], f32r, name="xiT")
        for n in range(SH_NI // 512):
            xp = ps_sm.tile([SIZE, 512], f32, tag="sm", name="ximm")
            for b in range(2):
                nc.tensor.matmul(out=xp[:], lhsT=gW2T_32[b][:],
                                 rhs=h2a[b][:, n * 512:(n + 1) * 512],
                                 start=(b == 0), stop=(b == 1))
            nc.vector.tensor_scalar_add(out=xiT[:, n * 512:(n + 1) * 512],
                                        in0=xp[:], scalar1=g_b2[:])
        xin = sb.tile([128, SH_NI // 128, SIZE], f32r, name="xin")
        for g in range(SH_NI // 128):
            tp = ps_sm.tile([128, SIZE], f32r, tag="sm", name="xi_tp")
            nc.tensor.transpose(out=tp[:], in_=xiT[:, g * 128:(g + 1) * 128],
                                identity=identr[:SIZE, :SIZE])
            nc.scalar.copy(out=xin[:, g, :], in_=tp[:])
        praw = ps_acc.tile([SIZE, SIZE], f32, tag="acc", name="praw")
        for g in range(SH_NI // 128):
            nc.tensor.matmul(out=praw[:], lhsT=xin[:, g, :], rhs=xin[:, g, :],
                             start=(g == 0), stop=(g == SH_NI // 128 - 1))
        nc.scalar.copy(out=pay3[:SIZE, 0:SIZE], in_=praw[:])
        nc.vector.reduce_sum(out=pay3[:SIZE, SIZE:SIZE + 1], in_=xiT[:], axis=AX.X)

        # ---------------- AG3 + combine
        ag3_in = dram.tile([128, AG3F], f32, name="ag3_in")
        nc.sync.dma_start(out=ag3_in[:], in_=pay3[:])
        nc.gpsimd.collective_compute(
            "AllGather", ALU.bypass, ins=[ag3_in[:].opt()],
            outs=[ag3_out[:].opt()], replica_groups=[list(range(NCORES))])
        ag3l = sb.tile([128, AG3F, NCORES], f32, name="ag3l")
        nc.sync.dma_start(out=ag3l[:],
                          in_=ag3_out[:].rearrange("(c p) f -> p f c", p=128))
        sum3 = sb.tile([128, NADD], f32, name="sum3")
        nc.vector.reduce_sum(out=sum3[:], in_=ag3l[:, 0:NADD, :], axis=AX.X)
        dmin = sb.tile([128, 32], f32, name="dmin")
        nc.vector.tensor_reduce(out=dmin[:], in_=ag3l[:, NADD:AG3F, :],
                                axis=AX.X, op=ALU.min)
        dmin16 = sb.tile([128, 16], f32, name="dmin16")
        dmv = dmin[:].rearrange("p (i h) -> p i h", h=2)
        nc.vector.tensor_tensor(out=dmin16[:], in0=dmv[:, :, 0], in1=dmv[:, :, 1],
                                op=ALU.min)
        dsum = sb.tile([128, 1], f32, name="dsum")
        nc.vector.reduce_sum(out=dsum[:], in_=dmin16[:], axis=AX.X)

        # ---------------- final assembly (fp32 [64,64])
        S64 = SIZE

        def new64(tag):
            return sb.tile([S64, S64], f32, tag=tag, name=tag)

        csum = sb.tile([S64, 1], f32, name="csum")
        nc.vector.tensor_copy(out=csum[:], in_=sum3[:S64, S64:S64 + 1])
        cr_ps = ps_sm.tile([1, S64], f32, tag="sm", name="cr_ps")
        nc.tensor.transpose(out=cr_ps[:], in_=csum[:], identity=ident_32[:S64, :S64])
        csr = sb.tile([1, S64], f32, name="csr")
        nc.scalar.copy(out=csr[:], in_=cr_ps[:])
        mr = sb.tile([1, S64], f32, name="mr")
        nc.scalar.activation(out=mr[:], in_=csr[:], func=AF.Copy, scale=1.0 / NIND)
        outer_ps = ps_sm.tile([S64, S64], f32, tag="sm", name="outer_ps")
        nc.tensor.matmul(out=outer_ps[:], lhsT=mr[:], rhs=csr[:],
                         start=True, stop=True)
        S_t = new64("S_t")
        nc.vector.tensor_tensor(out=S_t[:], in0=sum3[:S64, 0:S64], in1=outer_ps[:],
                                op=ALU.subtract)
        dtmp = new64("dtmp")
        nc.vector.tensor_tensor(out=dtmp[:], in0=S_t[:], in1=eye[:], op=ALU.mult)
        s2 = sb.tile([S64, 1], f32, name="s2")
        nc.vector.reduce_sum(out=s2[:], in_=dtmp[:], axis=AX.X)
        r2 = sb.tile([S64, 1], f32, name="r2")
        nc.vector.reciprocal(out=r2[:], in_=s2[:])
        s2r_ps = ps_sm.tile([1, S64], f32, tag="sm", name="s2r_ps")
        nc.tensor.transpose(out=s2r_ps[:], in_=s2[:], identity=ident_32[:S64, :S64])
        s2row = sb.tile([1, S64], f32, name="s2row")
        nc.scalar.copy(out=s2row[:], in_=s2r_ps[:])
        onesr64 = sb.tile([1, S64], f32, tag="onesr64", name="onesr64")
        nc.vector.memset(onesr64[:], 1.0)
        s2b_ps = ps_sm.tile([S64, S64], f32, tag="sm", name="s2b_ps")
        nc.tensor.matmul(out=s2b_ps[:], lhsT=onesr64[:], rhs=s2row[:],
                         start=True, stop=True)
        s2b = new64("s2b")
        nc.scalar.copy(out=s2b[:], in_=s2b_ps[:])
        SS = new64("SS")
        nc.vector.tensor_tensor(out=SS[:], in0=S_t[:], in1=S_t[:], op=ALU.mult)
        F_t = new64("F_t")
        nc.vector.tensor_scalar_mul(out=F_t[:], in0=SS[:], scalar1=r2[:])
        dg = new64("dg")
        nc.vector.tensor_tensor(out=dg[:], in0=s2b[:], in1=F_t[:], op=ALU.subtract)
        nc.vector.tensor_tensor(out=dg[:], in0=dg[:], in1=eye[:], op=ALU.add)
        B_t = new64("B_t")
        nc.vector.reciprocal(out=B_t[:], in_=dg[:])
        nc.vector.tensor_tensor(out=B_t[:], in0=B_t[:], in1=offd[:], op=ALU.mult)
        C_t = new64("C_t")
        nc.vector.tensor_tensor(out=C_t[:], in0=Lc[:], in1=LTc[:], op=ALU.subtract)
        nc.scalar.activation(out=C_t[:], in_=C_t[:], func=AF.Sigmoid)
        nc.vector.tensor_tensor(out=C_t[:], in0=C_t[:], in1=offd[:], op=ALU.mult)
        CT_t = new64("CT_t")
        nc.vector.tensor_tensor(out=CT_t[:], in0=LTc[:], in1=Lc[:], op=ALU.subtract)
        nc.scalar.activation(out=CT_t[:], in_=CT_t[:], func=AF.Sigmoid)
        nc.vector.tensor_tensor(out=CT_t[:], in0=CT_t[:], in1=offd[:], op=ALU.mult)
        U_t = new64("U_t")
        nc.vector.tensor_tensor(out=U_t[:], in0=CT_t[:], in1=C_t[:], op=ALU.add)
        cc_ps = ps_sm.tile([S64, S64], f32, tag="sm", name="cc_ps")
        nc.tensor.matmul(out=cc_ps[:], lhsT=CT_t[:], rhs=C_t[:],
                         start=True, stop=True)
        lt_t = new64("lt_t")
        nc.vector.tensor_tensor(out=lt_t[:], in0=cc_ps[:], in1=CT_t[:], op=ALU.mult)
        fin64 = sb.tile([S64, 8], f32, name="fin64")
        nc.vector.reduce_sum(out=fin64[:, 0:1], in_=lt_t[:], axis=AX.X)
        P_t = new64("P_t")
        nc.vector.tensor_tensor(out=P_t[:], in0=U_t[:], in1=B_t[:], op=ALU.mult)
        Q_t = new64("Q_t")
        nc.vector.tensor_tensor(out=Q_t[:], in0=C_t[:], in1=B_t[:], op=ALU.mult)
        ptq_ps = ps_sm.tile([S64, S64], f32, tag="sm", name="ptq_ps")
        nc.tensor.matmul(out=ptq_ps[:], lhsT=P_t[:], rhs=Q_t[:],
                         start=True, stop=True)
        t1_t = new64("t1_t")
        nc.vector.tensor_tensor(out=t1_t[:], in0=SS[:], in1=ptq_ps[:], op=ALU.mult)
        nc.vector.reduce_sum(out=fin64[:, 1:2], in_=t1_t[:], axis=AX.X)
        A_t = new64("A_t")
        nc.vector.tensor_tensor(out=A_t[:], in0=P_t[:], in1=S_t[:], op=ALU.mult)
        Bt_t = new64("Bt_t")
        nc.vector.tensor_tensor(out=Bt_t[:], in0=Q_t[:], in1=S_t[:], op=ALU.mult)
        nc.vector.tensor_scalar_mul(out=Bt_t[:], in0=Bt_t[:], scalar1=r2[:])
        ab_ps = ps_sm.tile([S64, S64], f32, tag="sm", name="ab_ps")
        nc.tensor.matmul(out=ab_ps[:], lhsT=A_t[:], rhs=Bt_t[:],
                         start=True, stop=True)
        t2_t = new64("t2_t")
        nc.vector.tensor_tensor(out=t2_t[:], in0=S_t[:], in1=ab_ps[:], op=ALU.mult)
        nc.vector.reduce_sum(out=fin64[:, 2:3], in_=t2_t[:], axis=AX.X)
        g1 = new64("t1_t")
        nc.vector.tensor_tensor(out=g1[:], in0=P_t[:], in1=SS[:], op=ALU.mult)
        gc = sb.tile([S64, 1], f32, tag="gcol", name="gcol")
        nc.vector.reduce_sum(out=gc[:], in_=g1[:], axis=AX.X)
        d1 = new64("t2_t")
        nc.vector.tensor_tensor(out=d1[:], in0=Q_t[:], in1=SS[:], op=ALU.mult)
        dc = sb.tile([S64, 1], f32, tag="dcol", name="dcol")
        nc.vector.reduce_sum(out=dc[:], in_=d1[:], axis=AX.X)
        t3c = sb.tile([S64, 1], f32, tag="t3col", name="t3col")
        nc.vector.tensor_tensor(out=t3c[:], in0=gc[:], in1=dc[:], op=ALU.mult)
        nc.vector.tensor_tensor(out=t3c[:], in0=t3c[:], in1=r2[:], op=ALU.mult)
        nc.vector.tensor_tensor(out=t3c[:], in0=t3c[:], in1=r2[:], op=ALU.mult)
        nc.vector.tensor_copy(out=fin64[:, 3:4], in_=t3c[:])
        t4_t = new64("lt_t")
        nc.vector.tensor_tensor(out=t4_t[:], in0=U_t[:], in1=C_t[:], op=ALU.mult)
        nc.vector.reduce_sum(out=fin64[:, 4:5], in_=t4_t[:], axis=AX.X)
        r2b = new64("dtmp")
        nc.vector.reciprocal(out=r2b[:], in_=s2b[:])
        ss_t = new64("t1_t")
        nc.vector.tensor_tensor(out=ss_t[:], in0=F_t[:], in1=r2b[:], op=ALU.mult)
        nc.vector.tensor_tensor(out=ss_t[:], in0=ss_t[:], in1=offd[:], op=ALU.mult)
        nc.vector.reduce_sum(out=fin64[:, 5:6], in_=ss_t[:], axis=AX.X)
        nc.vector.tensor_copy(out=fin64[:, 6:7], in_=sum3[:S64, 65:66])
        nc.vector.memset(fin64[:, 7:8], 0.0)

        f64_ps = ps_sm.tile([1, 8], f32, tag="sm", name="f64_ps")
        nc.tensor.matmul(out=f64_ps[:], lhsT=ones64[:], rhs=fin64[:],
                         start=True, stop=True)
        frow = sb.tile([1, 8], f32, name="frow")
        nc.scalar.copy(out=frow[:], in_=f64_ps[:])
        fin128 = sb.tile([128, 2], f32, name="fin128")
        nc.vector.tensor_copy(out=fin128[:, 0:1], in_=dsum[:])
        nc.vector.tensor_copy(out=fin128[:, 1:2], in_=zpsq_col[:])
        f128_ps = ps_sm.tile([1, 2], f32, tag="sm", name="f128_ps")
        nc.tensor.matmul(out=f128_ps[:], lhsT=ones128[:], rhs=fin128[:],
                         start=True, stop=True)
        grow = sb.tile([1, 2], f32, name="grow")
        nc.scalar.copy(out=grow[:], in_=f128_ps[:])

        acc = sb.tile([1, 1], f32, name="acc_sc")
        tmp = sb.tile([1, 1], f32, tag="tmp_sc", name="tmp_sc")
        nc.vector.tensor_copy(out=acc[:], in_=frow[:, 0:1])
        nc.scalar.activation(out=tmp[:], in_=frow[:, 6:7], func=AF.Copy,
                             scale=1.0 / (NS * SIZE))
        nc.vector.tensor_tensor(out=acc[:], in0=acc[:], in1=tmp[:], op=ALU.add)
        nc.scalar.activation(out=tmp[:], in_=grow[:, 0:1], func=AF.Copy,
                             scale=1.0 / (BTR * LAT))
        nc.vector.tensor_tensor(out=acc[:], in0=acc[:], in1=tmp[:], op=ALU.add)
        nc.scalar.activation(out=tmp[:], in_=grow[:, 1:2], func=AF.Copy,
                             scale=0.25 / (BTR * LAT))
        nc.vector.tensor_tensor(out=acc[:], in0=acc[:], in1=tmp[:], op=ALU.add)
        nc.vector.tensor_tensor(out=acc[:], in0=acc[:], in1=frow[:, 1:2],
                                op=ALU.add)
        nc.scalar.activation(out=tmp[:], in_=frow[:, 2:3], func=AF.Copy,
                             scale=-2.0)
        nc.vector.tensor_tensor(out=acc[:], in0=acc[:], in1=tmp[:], op=ALU.add)
        nc.vector.tensor_tensor(out=acc[:], in0=acc[:], in1=frow[:, 3:4],
                                op=ALU.add)
        nc.vector.tensor_tensor(out=acc[:], in0=acc[:], in1=frow[:, 4:5],
                                op=ALU.subtract)
        nc.scalar.activation(out=tmp[:], in_=frow[:, 5:6], func=AF.Copy,
                             scale=float(S64 - 2))
        nc.vector.tensor_tensor(out=acc[:], in0=acc[:], in1=tmp[:], op=ALU.add)
        nc.sync.dma_start(out=out_d[:], in_=acc[:])

    _split_multi_waits(nc)
    return nc


def _stage_inputs(I):
    g = lambda k: np.asarray(I[k], dtype=np.float32)
    z = g("z_logits")
    X = g("X")
    ntr = g("noise_trans")
    nind = g("noise_indep")
    perm = np.asarray(I["perm_idx"], dtype=np.int32).reshape(-1)
    L = g("conn_logits")

    def bf(a):
        return np.ascontiguousarray(a.astype(bfnp))

    def f(a):
        return np.ascontiguousarray(a.astype(np.float32))

    z_e32 = np.concatenate([z, np.ones((NS, 1), np.float32)], axis=1)
    shared = {
        "zext": bf(z_e32),
        "ntrT": bf(ntr.T),
        "ntr_ext": bf(np.concatenate([ntr, np.ones((BTR, 1), np.float32)], 1)),
        "nind_e32": f(np.concatenate([nind, np.ones((NIND, 1), np.float32)], 1)),
        "gW1T_bf": bf(g("glo_W1").T), "gW1nat_bf": bf(g("glo_W1")),
        "gW1T_32": f(g("glo_W1").T),
        "gW2T_bf": bf(g("glo_W2").T), "gW2T_32": f(g("glo_W2").T),
        "tW1T_bf": bf(g("tr_W1").T), "tW1nat_bf": bf(g("tr_W1")),
        "tW1T_32": f(g("tr_W1").T),
        "tW2T_bf": bf(g("tr_W2").T), "tW2T_32": f(g("tr_W2").T),
        "g_gam": f(g("glo_gamma").reshape(-1, 1)),
        "g_bet": f(g("glo_beta").reshape(-1, 1)),
        "t_gam": f(g("tr_gamma").reshape(-1, 1)),
        "t_bet": f(g("tr_beta").reshape(-1, 1)),
        "g_b2": f(g("glo_b2").reshape(-1, 1)),
        "t_b2": f(g("tr_b2").reshape(-1, 1)),
        "L32": f(L), "LT32": f(L.T),
        "eye64": np.eye(SIZE, dtype=np.float32),
        "offd64": (1.0 - np.eye(SIZE)).astype(np.float32),
        "ident_bf": np.eye(128, dtype=bfnp),
        "ident_32": np.eye(128, dtype=np.float32),
        "identr": np.eye(128, dtype=np.float32),
        "ones_row_bf": np.ones((1, 128), bfnp),
        "ones_col_bf": np.ones((128, 1), bfnp),
        "ones64_32": np.ones((SIZE, 1), np.float32),
        "ones128_32": np.ones((128, 1), np.float32),
    }
    zT = z.T
    XT = X.T
    nindT = nind.T
    maps = []
    for c in range(NCORES):
        m = dict(shared)
        m["znat32"] = f(z_e32[c * SH_NS:(c + 1) * SH_NS, :])
        m["zT_sh"] = bf(zT[:, c * SH_NS:(c + 1) * SH_NS])
        m["xT_sh"] = bf(XT[:, c * SH_NS:(c + 1) * SH_NS])
        m["nindT32"] = f(nindT[:, c * SH_NI:(c + 1) * SH_NI])
        m["perm_sh"] = np.ascontiguousarray(
            perm[c * SH_J:(c + 1) * SH_J].reshape(-1, 1))
        maps.append(m)
    return maps


def _get_nc():
    if "nc" not in _CACHE:
        _install_profshim()
        _CACHE["nc"] = _build_program()
    return _CACHE["nc"]


def run(inputs, trace=False):
    nc = _get_nc()
    maps = _stage_inputs(inputs)
    res = run_bass_kernel_spmd(nc, maps, list(range(NCORES)), trace=trace)
    val = np.float32(res.results[0]["out"].reshape(-1)[0])
    return val, res


def kernel(**inputs) -> np.ndarray:
    val, _ = run(inputs, trace=False)
    return np.asarray(val, dtype=np.float32)


if __name__ == "__main__":
    nc = _get_nc()
    ninst = sum(len(bb.instructions) for bb in nc.main_func.blocks)
    print("built ok, instructions:", ninst)


# revision 14
# speedup vs baseline: 1.8493x; 1.8493x over previous
"""Trainium2 Bass kernel for nn_CausalityChainModel (loss_fn), 8-core SPMD.

Self-contained: takes FULL inputs, shards internally across 8 NeuronCores,
runs one Bass/Tile program via run_bass_kernel_spmd, returns the scalar loss.

Key math (validated numerically against the reference on CPU):
- loss_indep's [n,N,n] residual tensor collapses analytically:
      G[j,i,k] = S[i,k] - S[j,i]S[j,k]/s2[j]
  (S = centered Gram of X_ind), and the masked weighted triple sum reduces to
  a handful of [64,64] matrix products.
- BatchNorm (train-mode, biased var) stats come from raw Gram matrices of the
  layer inputs: E[h] = W1 colsum(x)/N, E[h^2] = diag(W1 G W1^T)/N, G = x^T x.
  So BN+LeakyReLU is one ACT pass: Lrelu(psum*scale + bias).
- Large matmuls in bf16; the X_ind chain in float32r (full-rate, ~1e-3 rel);
  the Grams feeding X_ind-path BN stats in full fp32.
- Sharding: sample axes of z_logits/X/noise_indep split across cores;
  noise_trans (Zp) replicated; NCT candidates (Zs) sharded over j with a
  min-combine in the final AllGather.
- Collectives (AllGather only, queued in order): AG1 gram-z partials (first,
  absorbs the ~70us ncfw cold-start under local compute), AG2 X_ind-h BN stat
  sums, AG3 S-gram/colsum/mse partials + per-i distance mins.
"""
import os
import sys
import types
import contextlib

for _p in ("/opt/trn_rl_repo", "/root/.axon_site"):
    if _p not in sys.path:
        sys.path.insert(0, _p)

import numpy as np
import ml_dtypes

import concourse.bass as bass
import concourse.tile as tile
from concourse import mybir
from concourse.bass_utils import run_bass_kernel_spmd

SIZE, NS, LAT, NOISE, HID, BTR, NIND = 64, 16384, 128, 64, 256, 2048, 8192
NCORES = 8
SH_NS = NS // NCORES      # 2048
SH_NI = NIND // NCORES    # 1024
SH_J = NS // NCORES       # 2048 Zs rows per core
BN_EPS = 1e-5
LRELU = 0.01

f32 = mybir.dt.float32
f32r = mybir.dt.float32r
bf16 = mybir.dt.bfloat16
i32 = mybir.dt.int32
AF = mybir.ActivationFunctionType
ALU = mybir.AluOpType
AX = mybir.AxisListType
bfnp = ml_dtypes.bfloat16

AG1F = LAT + 1            # gram-z partial columns
AG2F = 8                  # sum(h2) x4 chunks, sum(h2^2) x4 chunks
AG3F = 98                 # 0-63 S, 64 colsum, 65 mse, 66-97 dmin (32 cols)
NADD = 66
NI_CH = 16
BIGF = 3.0e38

_CACHE = {}


def _install_profshim():
    if "antenv.axon_hooks" in sys.modules:
        return
    try:
        import antenv
        mod = types.ModuleType("antenv.axon_hooks")
        mod._hook = None
        mod.set_axon_ntff_profile_hook = lambda h: setattr(mod, "_hook", h)
        mod.get_axon_ntff_profile_hook = lambda: mod._hook
        sys.modules["antenv.axon_hooks"] = mod
        antenv.axon_hooks = mod
        from trn_agent_boot import trn_boot
        so = "/opt/axon/libaxon_pjrt.so"
        if os.path.exists(so):
            mod.set_axon_ntff_profile_hook(trn_boot._ntff_profile_via_ctypes(so))
        import concourse.bass_utils as bu
        bu.upload_artifacts = lambda tmpdir: str(tmpdir)
    except Exception:
        pass


def _split_multi_waits(nc, max_waits=1):
    """This walrus build rejects >1 sem-wait per instruction: move extras onto
    EventSemaphore nops (cheap, non-pipeline-flushing) placed just before."""
    for bb in nc.main_func.blocks:
        new_insts = []
        for inst in bb.instructions:
            si = inst.sync_info
            if si is not None and len(si.on_wait) > max_waits:
                waits = list(si.on_wait)
                extra, keep = waits[:-max_waits], waits[-max_waits:]
                for i in range(0, len(extra), max_waits):
                    d = mybir.InstEventSemaphore(
                        name=f"{inst.name}-wsplit{i}", ins=[], outs=[])
                    d.engine = inst.engine
                    d.sync_info = mybir.SyncInfo(
                        on_wait=list(extra[i:i + max_waits]), on_update=[])
                    new_insts.append(d)
                inst.sync_info = mybir.SyncInfo(
                    on_wait=list(keep), on_update=list(si.on_update))
            new_insts.append(inst)
        try:
            bb.instructions[:] = new_insts
        except TypeError:
            bb.instructions = new_insts


def _build_program():
    nc = bass.Bass()

    def din(name, shape, dt):
        return nc.dram_tensor(name, shape, dt, kind="ExternalInput")

    zext = din("zext", [NS, LAT + 1], bf16)            # gather source
    znat32 = din("znat32", [SH_NS, LAT + 1], f32)      # shard, z|ones fp32
    zT_sh = din("zT_sh", [LAT, SH_NS], bf16)
    xT_sh = din("xT_sh", [SIZE, SH_NS], bf16)
    ntrT = din("ntrT", [NOISE, BTR], bf16)
    ntr_ext = din("ntr_ext", [BTR, NOISE + 1], bf16)
    nind_e32 = din("nind_e32", [NIND, NOISE + 1], f32)
    nindT32 = din("nindT32", [NOISE, SH_NI], f32r)
    perm_sh = din("perm_sh", [128, NI_CH], i32)
    gW1T_bf_d = din("gW1T_bf", [LAT, HID], bf16)
    gW1nat_d = din("gW1nat_bf", [HID, LAT], bf16)
    gW2T_bf_d = din("gW2T_bf", [HID, SIZE], bf16)
    gW2T_32_d = din("gW2T_32", [HID, SIZE], f32r)
    gW1T_32_d = din("gW1T_32", [LAT, HID], f32r)
    tW1T_bf_d = din("tW1T_bf", [NOISE, HID], bf16)
    tW1nat_d = din("tW1nat_bf", [HID, NOISE], bf16)
    tW1T_32_d = din("tW1T_32", [NOISE, HID], f32r)
    tW2T_bf_d = din("tW2T_bf", [HID, LAT], bf16)
    tW2T_32_d = din("tW2T_32", [HID, LAT], f32r)
    g_gam_d = din("g_gam", [HID, 1], f32)
    g_bet_d = din("g_bet", [HID, 1], f32)
    t_gam_d = din("t_gam", [HID, 1], f32)
    t_bet_d = din("t_bet", [HID, 1], f32)
    g_b2_d = din("g_b2", [SIZE, 1], f32)
    t_b2_d = din("t_b2", [LAT, 1], f32)
    L32_d = din("L32", [SIZE, SIZE], f32)
    LT32_d = din("LT32", [SIZE, SIZE], f32)
    eye64_d = din("eye64", [SIZE, SIZE], f32)
    offd64_d = din("offd64", [SIZE, SIZE], f32)
    ident_bf_d = din("ident_bf", [128, 128], bf16)
    ident_32_d = din("ident_32", [128, 128], f32)
    identr_d = din("identr", [128, 128], f32r)
    ones_row_d = din("ones_row_bf", [1, 128], bf16)
    ones_col_d = din("ones_col_bf", [128, 1], bf16)
    ones64_d = din("ones64_32", [SIZE, 1], f32)
    ones128_d = din("ones128_32", [128, 1], f32)

    out_d = nc.dram_tensor("out", [1, 1], f32, kind="ExternalOutput")

    ag1_out = nc.dram_tensor("ag1_out", [NCORES * 128, AG1F], f32,
                             addr_space="Shared")
    ag2_out = nc.dram_tensor("ag2_out", [NCORES * 128, AG2F], f32,
                             addr_space="Shared")
    ag3_out = nc.dram_tensor("ag3_out", [NCORES * 128, AG3F], f32,
                             addr_space="Shared")

    with tile.TileContext(nc) as tc, contextlib.ExitStack() as ctx:
        const = ctx.enter_context(tc.tile_pool(name="const", bufs=1))
        sb = ctx.enter_context(tc.tile_pool(name="sb", bufs=1))
        sb3 = ctx.enter_context(tc.tile_pool(name="sb3", bufs=4))
        ps_acc = ctx.enter_context(tc.tile_pool(name="ps_acc", bufs=2, space="PSUM"))
        ps_sm = ctx.enter_context(tc.tile_pool(name="ps_sm", bufs=2, space="PSUM"))
        ps_d = ctx.enter_context(tc.tile_pool(name="ps_d", bufs=2, space="PSUM"))
        dram = ctx.enter_context(tc.tile_pool(name="dram", bufs=1, space="DRAM"))

        # ---------------- input loads
        def load(shape, dt, src, name):
            t = const.tile(shape, dt, tag=name, name=name)
            nc.sync.dma_start(out=t[:], in_=src)
            return t

        def load2(shape, dt, src_d, name):
            return [load([128, shape[1]], dt,
                         src_d[b * 128:(b + 1) * 128, :], f"{name}{b}")
                    for b in range(2)]

        t_znat = sb.tile([128, SH_NS // 128, LAT + 1], f32, name="t_znat")
        nc.sync.dma_start(out=t_znat[:],
                          in_=znat32[:].rearrange("(c p) f -> p c f", p=128))

        ident_bf = load([128, 128], bf16, ident_bf_d[:], "ident_bf")
        ident_32 = load([128, 128], f32, ident_32_d[:], "ident_32")
        identr = load([128, 128], f32r, identr_d[:], "identr")
        ones_row = load([1, 128], bf16, ones_row_d[:], "ones_row")
        ones_col = load([128, 1], bf16, ones_col_d[:], "ones_col")
        ones64 = load([SIZE, 1], f32, ones64_d[:], "ones64")
        ones128 = load([128, 1], f32, ones128_d[:], "ones128")
        eps_col = const.tile([128, 1], f32, tag="eps_col", name="eps_col")
        nc.vector.memset(eps_col[:], BN_EPS)
        eye = load([SIZE, SIZE], f32, eye64_d[:], "eye")
        offd = load([SIZE, SIZE], f32, offd64_d[:], "offd")
        Lc = load([SIZE, SIZE], f32, L32_d[:], "L")
        LTc = load([SIZE, SIZE], f32, LT32_d[:], "LT")
        gW1T_bf = load([LAT, HID], bf16, gW1T_bf_d[:], "gW1T_bf")
        gW1nat = load2([HID, LAT], bf16, gW1nat_d, "gW1nat")
        gW2T_bf = load2([HID, SIZE], bf16, gW2T_bf_d, "gW2T_bf")
        gW2T_32 = load2([HID, SIZE], f32r, gW2T_32_d, "gW2T_32")
        gW1T_32 = load([LAT, HID], f32r, gW1T_32_d[:], "gW1T_32")
        tW1T_bf = load([NOISE, HID], bf16, tW1T_bf_d[:], "tW1T_bf")
        tW1nat = load2([HID, NOISE], bf16, tW1nat_d, "tW1nat")
        tW1T_32 = load([NOISE, HID], f32r, tW1T_32_d[:], "tW1T_32")
        tW2T_bf = load2([HID, LAT], bf16, tW2T_bf_d, "tW2T_bf")
        tW2T_32 = load2([HID, LAT], f32r, tW2T_32_d, "tW2T_32")
        g_b2 = load([SIZE, 1], f32, g_b2_d[:], "g_b2")
        t_b2 = load([LAT, 1], f32, t_b2_d[:], "t_b2")
        g_gam = [load([128, 1], f32, g_gam_d[b * 128:(b + 1) * 128, :], f"g_gam{b}")
                 for b in range(2)]
        g_bet = [load([128, 1], f32, g_bet_d[b * 128:(b + 1) * 128, :], f"g_bet{b}")
                 for b in range(2)]
        t_gam = [load([128, 1], f32, t_gam_d[b * 128:(b + 1) * 128, :], f"t_gam{b}")
                 for b in range(2)]
        t_bet = [load([128, 1], f32, t_bet_d[b * 128:(b + 1) * 128, :], f"t_bet{b}")
                 for b in range(2)]

        t_zT = sb.tile([LAT, SH_NS], bf16, name="t_zT")
        nc.sync.dma_start(out=t_zT[:], in_=zT_sh[:])
        t_xT = sb.tile([SIZE, SH_NS], bf16, name="t_xT")
        nc.sync.dma_start(out=t_xT[:], in_=xT_sh[:])
        t_ntrT = sb.tile([NOISE, BTR], bf16, name="t_ntrT")
        nc.sync.dma_start(out=t_ntrT[:], in_=ntrT[:])
        t_nindT = sb.tile([NOISE, SH_NI], f32r, name="t_nindT")
        nc.sync.dma_start(out=t_nindT[:], in_=nindT32[:])
        t_perm = sb.tile([128, NI_CH], i32, name="t_perm")
        nc.sync.dma_start(out=t_perm[:], in_=perm_sh[:])

        # ---------------- AG1: sharded fp32 gram of z (16 fp32 matmuls)
        gz_ps = ps_acc.tile([LAT, AG1F], f32, tag="acc", name="gz_ps")
        for k in range(SH_NS // 128):
            nc.tensor.matmul(out=gz_ps[:], lhsT=t_znat[:, k, :LAT],
                             rhs=t_znat[:, k, :],
                             start=(k == 0), stop=(k == SH_NS // 128 - 1))
        pay1 = sb.tile([128, AG1F], f32, name="pay1")
        nc.scalar.copy(out=pay1[:], in_=gz_ps[:])
        ag1_in = dram.tile([128, AG1F], f32, name="ag1_in")
        nc.sync.dma_start(out=ag1_in[:], in_=pay1[:])
        nc.gpsimd.collective_compute(
            "AllGather", ALU.bypass, ins=[ag1_in[:].opt()],
            outs=[ag1_out[:].opt()], replica_groups=[list(range(NCORES))])

        # ---------------- replicated grams: noise_trans (bf16), noise_ind (fp32)
        def gram_from_dram(src, dtype, nrows, nin, tag):
            nch = nrows // 128
            grp = 8
            gps = ps_acc.tile([nin, nin + 1], f32, tag="acc", name=f"g_{tag}")
            view = src[:].rearrange("(c p) f -> p c f", p=128)
            for k0 in range(0, nch, grp):
                stage = sb3.tile([128, grp, nin + 1], dtype, tag=f"gs_{tag}",
                                 name=f"gs_{tag}")
                nc.sync.dma_start(out=stage[:], in_=view[:, k0:k0 + grp, :])
                for j in range(grp):
                    k = k0 + j
                    nc.tensor.matmul(out=gps[:], lhsT=stage[:, j, :nin],
                                     rhs=stage[:, j, :],
                                     start=(k == 0), stop=(k == nch - 1))
            gsb = sb.tile([nin, nin + 1], f32, tag=f"gss_{tag}", name=f"gss_{tag}")
            nc.scalar.copy(out=gsb[:], in_=gps[:])
            return gsb

        gtr = gram_from_dram(ntr_ext, bf16, BTR, NOISE, "tr")
        gni = gram_from_dram(nind_e32, f32, NIND, NOISE, "ni")

        # ---------------- BN stats from a Gram
        def _stat_tail(esq_or_tot2, mu, gam, bet, N, tag):
            var = sb.tile([128, 1], f32, tag=f"var_{tag}", name=f"var_{tag}")
            nc.scalar.activation(out=var[:], in_=esq_or_tot2[:], func=AF.Copy,
                                 scale=1.0 / N)
            musq = sb.tile([128, 1], f32, tag="stat_musq", name="stat_musq")
            nc.vector.tensor_tensor(out=musq[:], in0=mu[:], in1=mu[:], op=ALU.mult)
            nc.vector.tensor_tensor(out=var[:], in0=var[:], in1=musq[:],
                                    op=ALU.subtract)
            std = sb.tile([128, 1], f32, tag="stat_std", name="stat_std")
            nc.scalar.activation(out=std[:], in_=var[:], func=AF.Sqrt,
                                 bias=eps_col[:])
            rstd = sb.tile([128, 1], f32, tag="stat_rstd", name="stat_rstd")
            nc.vector.reciprocal(out=rstd[:], in_=std[:])
            s = sb.tile([128, 1], f32, tag=f"s_{tag}", name=f"s_{tag}")
            nc.vector.tensor_tensor(out=s[:], in0=gam[:], in1=rstd[:], op=ALU.mult)
            bb_ = sb.tile([128, 1], f32, tag=f"b_{tag}", name=f"b_{tag}")
            nc.vector.tensor_tensor(out=bb_[:], in0=mu[:], in1=s[:], op=ALU.mult)
            nc.vector.tensor_tensor(out=bb_[:], in0=bet[:], in1=bb_[:],
                                    op=ALU.subtract)
            return s, bb_

        def stats_from_gram(gram, w1T, w1nat, gam, bet, n_in, N, tag,
                            use_bf=True):
            if use_bf:
                gmm = sb.tile([n_in, n_in + 1], bf16, tag=f"gb_{tag}",
                              name=f"gb_{tag}")
                nc.scalar.copy(out=gmm[:], in_=gram[:])
            else:
                gmm = gram
            scales, biases = [], []
            for b in range(2):
                mm = ps_acc.tile([128, n_in + 1], f32, tag="acc", name="stat_mm")
                nc.tensor.matmul(out=mm[:], lhsT=w1T[:, b * 128:(b + 1) * 128],
                                 rhs=gmm[:], start=True, stop=True)
                prod = sb.tile([128, n_in], f32, tag="stat_prod", name="stat_prod")
                nc.vector.tensor_tensor(out=prod[:], in0=mm[:, :n_in],
                                        in1=w1nat[b][:], op=ALU.mult)
                esq = sb.tile([128, 1], f32, tag=f"esq_{tag}{b}",
                              name=f"esq_{tag}{b}")
                nc.vector.reduce_sum(out=esq[:], in_=prod[:], axis=AX.X)
                mu = sb.tile([128, 1], f32, tag=f"mu_{tag}{b}", name=f"mu_{tag}{b}")
                nc.scalar.activation(out=mu[:], in_=mm[:, n_in:n_in + 1],
                                     func=AF.Copy, scale=1.0 / N)
                s, bias = _stat_tail(esq, mu, gam[b], bet[b], N, f"{tag}{b}")
                scales.append(s)
                biases.append(bias)
            return scales, biases

        tr_s, tr_b = stats_from_gram(gtr, tW1T_bf, tW1nat, t_gam, t_bet,
                                     NOISE, BTR, "tr")
        # stats matmuls for the ind path also in bf16 weights but fp32 gram:
        # mixed dtypes are not allowed -> cast gram to bf16 would lose the
        # fp32 gain; instead run these two stat matmuls in fp32.
        ind_s, ind_b = [], []
        for b in range(2):
            mm = ps_acc.tile([128, NOISE + 1], f32, tag="acc", name="istat_mm")
            # fp32 matmul: lhsT fp32 [64, 128], rhs fp32 [64, 65]
            tW1T_f = sb.tile([NOISE, 128], f32, tag=f"tW1Tf{b}", name=f"tW1Tf{b}")
            nc.vector.tensor_copy(out=tW1T_f[:], in_=tW1T_32[:, b * 128:(b + 1) * 128])
            nc.tensor.matmul(out=mm[:], lhsT=tW1T_f[:], rhs=gni[:],
                             start=True, stop=True)
            prod = sb.tile([128, NOISE], f32, tag="stat_prod", name="stat_prod")
            nc.vector.tensor_tensor(out=prod[:], in0=mm[:, :NOISE],
                                    in1=tW1nat[b][:], op=ALU.mult)
            esq = sb.tile([128, 1], f32, tag=f"esq_ind{b}", name=f"esq_ind{b}")
            nc.vector.reduce_sum(out=esq[:], in_=prod[:], axis=AX.X)
            mu = sb.tile([128, 1], f32, tag=f"mu_ind{b}", name=f"mu_ind{b}")
            nc.scalar.activation(out=mu[:], in_=mm[:, NOISE:NOISE + 1],
                                 func=AF.Copy, scale=1.0 / NIND)
            s, bias = _stat_tail(esq, mu, t_gam[b], t_bet[b], NIND, f"ind{b}")
            ind_s.append(s)
            ind_b.append(bias)

        # ---------------- tr branch: Zp (replicated), -2*(Zp+b2)
        h_tr = [sb.tile([128, BTR], bf16, tag=f"h_tr{b}", name=f"h_tr{b}")
                for b in range(2)]
        for b in range(2):
            for n in range(BTR // 512):
                hp = ps_sm.tile([128, 512], f32, tag="sm", name="hmm")
                nc.tensor.matmul(out=hp[:], lhsT=tW1T_bf[:, b * 128:(b + 1) * 128],
                                 rhs=t_ntrT[:, n * 512:(n + 1) * 512],
                                 start=True, stop=True)
                nc.scalar.activation(out=h_tr[b][:, n * 512:(n + 1) * 512],
                                     in_=hp[:], func=AF.Lrelu,
                                     bias=tr_b[b][:], scale=tr_s[b][:],
                                     alpha=LRELU)
        zpm2 = sb.tile([LAT, BTR], bf16, name="zpm2")
        for n in range(BTR // 512):
            zp = ps_sm.tile([LAT, 512], f32, tag="sm", name="zpmm")
            for b in range(2):
                nc.tensor.matmul(out=zp[:], lhsT=tW2T_bf[b][:],
                                 rhs=h_tr[b][:, n * 512:(n + 1) * 512],
                                 start=(b == 0), stop=(b == 1))
            nc.vector.tensor_scalar(out=zpm2[:, n * 512:(n + 1) * 512], in0=zp[:],
                                    scalar1=t_b2[:], scalar2=-2.0,
                                    op0=ALU.add, op1=ALU.mult)
        zpsq_scr = sb.tile([LAT, BTR], bf16, tag="sq128", name="zpsq_scr")
        zpsq_col = sb.tile([128, 1], f32, name="zpsq_col")
        nc.scalar.activation(out=zpsq_scr[:], in_=zpm2[:], func=AF.Square,
                             accum_out=zpsq_col[:])

        # ---------------- Zs gather + transpose + nsq broadcast rows
        zsT = sb.tile([LAT, SH_J], bf16, name="zsT")
        for g in range(NI_CH):
            gz_t = sb3.tile([128, LAT + 1], bf16, tag="zs_gather", name="zs_gather")
            nc.gpsimd.indirect_dma_start(
                out=gz_t[:], out_offset=None, in_=zext[:],
                in_offset=bass.IndirectOffsetOnAxis(ap=t_perm[:, g:g + 1], axis=0))
            tp = ps_sm.tile([128, 128], bf16, tag="sm", name="zs_tp")
            nc.tensor.transpose(out=tp[:], in_=gz_t[:, :LAT], identity=ident_bf[:])
            nc.scalar.copy(out=zsT[:, g * 128:(g + 1) * 128], in_=tp[:])
        zsq = sb.tile([LAT, SH_J], bf16, tag="sq128", name="zsq")
        nc.scalar.activation(out=zsq[:], in_=zsT[:], func=AF.Square)
        nsq_row = sb.tile([1, SH_J], bf16, name="nsq_row")
        for n in range(SH_J // 512):
            np_ = ps_sm.tile([1, 512], f32, tag="sm", name="nsqp")
            nc.tensor.matmul(out=np_[:], lhsT=ones_col[:],
                             rhs=zsq[:, n * 512:(n + 1) * 512],
                             start=True, stop=True)
            nc.scalar.copy(out=nsq_row[:, n * 512:(n + 1) * 512], in_=np_[:])

        # ---------------- ind chain (f32r): h_ind -> Z_ind -> h2 (+ stat sums)
        h_ind = [sb.tile([128, SH_NI], f32r, tag=f"h_ind{b}", name=f"h_ind{b}")
                 for b in range(2)]
        for b in range(2):
            for n in range(SH_NI // 512):
                hp = ps_sm.tile([128, 512], f32, tag="sm", name="himm")
                nc.tensor.matmul(out=hp[:], lhsT=tW1T_32[:, b * 128:(b + 1) * 128],
                                 rhs=t_nindT[:, n * 512:(n + 1) * 512],
                                 start=True, stop=True)
                nc.scalar.activation(out=h_ind[b][:, n * 512:(n + 1) * 512],
                                     in_=hp[:], func=AF.Lrelu,
                                     bias=ind_b[b][:], scale=ind_s[b][:],
                                     alpha=LRELU)
        ziT = sb.tile([LAT, SH_NI], f32r, name="ziT")
        for n in range(SH_NI // 512):
            zp = ps_sm.tile([LAT, 512], f32, tag="sm", name="zimm")
            for b in range(2):
                nc.tensor.matmul(out=zp[:], lhsT=tW2T_32[b][:],
                                 rhs=h_ind[b][:, n * 512:(n + 1) * 512],
                                 start=(b == 0), stop=(b == 1))
            nc.vector.tensor_scalar_add(out=ziT[:, n * 512:(n + 1) * 512],
                                        in0=zp[:], scalar1=t_b2[:])
        pay2 = sb.tile([128, AG2F], f32, name="pay2")
        h2 = [sb.tile([128, SH_NI], f32r, tag=f"h2_{b}", name=f"h2_{b}")
              for b in range(2)]
        sq_scr = sb.tile([128, 512], f32, tag="sqscr32", name="sq_scr")
        for b in range(2):
            for n in range(SH_NI // 512):
                hp = ps_sm.tile([128, 512], f32, tag="sm", name="h2mm")
                nc.tensor.matmul(out=hp[:], lhsT=gW1T_32[:, b * 128:(b + 1) * 128],
                                 rhs=ziT[:, n * 512:(n + 1) * 512],
                                 start=True, stop=True)
                col = b * 2 + n
                nc.scalar.activation(out=h2[b][:, n * 512:(n + 1) * 512],
                                     in_=hp[:], func=AF.Copy,
                                     accum_out=pay2[:, col:col + 1])
                nc.scalar.activation(out=sq_scr[:],
                                     in_=h2[b][:, n * 512:(n + 1) * 512],
                                     func=AF.Square,
                                     accum_out=pay2[:, 4 + col:5 + col])
        ag2_in = dram.tile([128, AG2F], f32, name="ag2_in")
        nc.sync.dma_start(out=ag2_in[:], in_=pay2[:])
        nc.gpsimd.collective_compute(
            "AllGather", ALU.bypass, ins=[ag2_in[:].opt()],
            outs=[ag2_out[:].opt()], replica_groups=[list(range(NCORES))])

        # ---------------- NCT distance loop (overlaps AG1/AG2)
        pay3 = sb.tile([128, AG3F], f32, name="pay3")
        nc.vector.memset(pay3[:], 0.0)
        for ic in range(NI_CH):
            for jh in range(2):
                dps = ps_d.tile([128, 1024], f32, tag="dps", name="dps")
                for jq in range(2):
                    off = jh * 1024 + jq * 512
                    sl = slice(jq * 512, (jq + 1) * 512)
                    nc.tensor.matmul(out=dps[:, sl], lhsT=ones_row[:],
                                     rhs=nsq_row[:, off:off + 512],
                                     start=True, stop=False)
                    nc.tensor.matmul(out=dps[:, sl],
                                     lhsT=zpm2[:, ic * 128:(ic + 1) * 128],
                                     rhs=zsT[:, off:off + 512],
                                     start=False, stop=True)
                col = NADD + ic * 2 + jh
                nc.vector.tensor_reduce(out=pay3[:, col:col + 1], in_=dps[:],
                                        axis=AX.X, op=ALU.min)

        # ---------------- AG1 combine -> glo stats -> glo branch -> mse
        ag1l = sb.tile([128, NCORES, AG1F], f32, name="ag1l")
        nc.sync.dma_start(out=ag1l[:],
                          in_=ag1_out[:].rearrange("(c p) f -> p c f", p=128))
        gz = sb.tile([128, AG1F], f32, name="gz")
        nc.vector.tensor_tensor(out=gz[:], in0=ag1l[:, 0, :], in1=ag1l[:, 1, :],
                                op=ALU.add)
        for c in range(2, NCORES):
            nc.vector.tensor_tensor(out=gz[:], in0=gz[:], in1=ag1l[:, c, :],
                                    op=ALU.add)
        glo_s, glo_b = stats_from_gram(gz, gW1T_bf, gW1nat, g_gam, g_bet,
                                       LAT, NS, "glo")
        h_glo = [sb.tile([128, SH_NS], bf16, tag=f"h_glo{b}", name=f"h_glo{b}")
                 for b in range(2)]
        for b in range(2):
            for n in range(SH_NS // 512):
                hp = ps_sm.tile([128, 512], f32, tag="sm", name="hgmm")
                nc.tensor.matmul(out=hp[:], lhsT=gW1T_bf[:, b * 128:(b + 1) * 128],
                                 rhs=t_zT[:, n * 512:(n + 1) * 512],
                                 start=True, stop=True)
                nc.scalar.activation(out=h_glo[b][:, n * 512:(n + 1) * 512],
                                     in_=hp[:], func=AF.Lrelu,
                                     bias=glo_b[b][:], scale=glo_s[b][:],
                                     alpha=LRELU)
        dtile = sb.tile([SIZE, SH_NS], f32, name="dtile")
        for n in range(SH_NS // 512):
            xp = ps_sm.tile([SIZE, 512], f32, tag="sm", name="xgmm")
            for b in range(2):
                nc.tensor.matmul(out=xp[:], lhsT=gW2T_bf[b][:],
                                 rhs=h_glo[b][:, n * 512:(n + 1) * 512],
                                 start=(b == 0), stop=(b == 1))
            nc.vector.scalar_tensor_tensor(
                out=dtile[:, n * 512:(n + 1) * 512], in0=xp[:], scalar=g_b2[:],
                in1=t_xT[:, n * 512:(n + 1) * 512], op0=ALU.add, op1=ALU.subtract)
        msesq = sb.tile([SIZE, SH_NS], bf16, tag="sq64", name="msesq")
        nc.scalar.activation(out=msesq[:], in_=dtile[:], func=AF.Square,
                             accum_out=pay3[:SIZE, 65:66])

        # ---------------- AG2 combine -> X_ind -> S partials
        ag2l = sb.tile([128, NCORES, AG2F], f32, name="ag2l")
        nc.sync.dma_start(out=ag2l[:],
                          in_=ag2_out[:].rearrange("(c p) f -> p c f", p=128))
        sums2 = sb.tile([128, AG2F], f32, name="sums2")
        nc.vector.tensor_tensor(out=sums2[:], in0=ag2l[:, 0, :],
                                in1=ag2l[:, 1, :], op=ALU.add)
        for c in range(2, NCORES):
            nc.vector.tensor_tensor(out=sums2[:], in0=sums2[:],
                                    in1=ag2l[:, c, :], op=ALU.add)
        h2_s, h2_b = [], []
        for b in range(2):
            tot = sb.tile([128, 1], f32, tag=f"h2tot{b}", name=f"h2tot{b}")
            nc.vector.tensor_tensor(out=tot[:], in0=sums2[:, 2 * b:2 * b + 1],
                                    in1=sums2[:, 2 * b + 1:2 * b + 2], op=ALU.add)
            mu = sb.tile([128, 1], f32, tag=f"h2mu{b}", name=f"h2mu{b}")
            nc.scalar.activation(out=mu[:], in_=tot[:], func=AF.Copy,
                                 scale=1.0 / NIND)
            tot2 = sb.tile([128, 1], f32, tag=f"h2tot2{b}", name=f"h2tot2{b}")
            nc.vector.tensor_tensor(out=tot2[:], in0=sums2[:, 4 + 2 * b:5 + 2 * b],
                                    in1=sums2[:, 5 + 2 * b:6 + 2 * b], op=ALU.add)
            s, bb_ = _stat_tail(tot2, mu, g_gam[b], g_bet[b], NIND, f"h2{b}")
            h2_s.append(s)
            h2_b.append(bb_)
        h2a = [sb.tile([128, SH_NI], f32r, tag=f"h2a{b}", name=f"h2a{b}")
               for b in range(2)]
        for b in range(2):
            nc.scalar.activation(out=h2a[b][:], in_=h2[b][:], func=AF.Lrelu,
                                 bias=h2_b[b][:], scale=h2_s[b][:], alpha=LRELU)
        xiT = sb.tile([SIZE, SH_NI], f32r, name="xiT")
        for n in range(SH_NI // 512):
            xp = ps_sm.tile([SIZE, 512], f32, tag="sm", name="ximm")
            for b in range(2):
                nc.tensor.matmul(out=xp[:], lhsT=gW2T_32[b][:],
                                 rhs=h2a[b][:, n * 512:(n + 1) * 512],
                                 start=(b == 0), stop=(b == 1))
            nc.vector.tensor_scalar_add(out=xiT[:, n * 512:(n + 1) * 512],
                                        in0=xp[:], scalar1=g_b2[:])
        xin = sb.tile([128, SH_NI // 128, SIZE], f32r, name="xin")
        for g in range(SH_NI // 128):
            tp = ps_sm.tile([128, SIZE], f32r, tag="sm", name="xi_tp")
            nc.tensor.transpose(out=tp[:], in_=xiT[:, g * 128:(g + 1) * 128],
                                identity=identr[:SIZE, :SIZE])
            nc.scalar.copy(out=xin[:, g, :], in_=tp[:])
        praw = ps_acc.tile([SIZE, SIZE], f32, tag="acc", name="praw")
        for g in range(SH_NI // 128):
            nc.tensor.matmul(out=praw[:], lhsT=xin[:, g, :], rhs=xin[:, g, :],
                             start=(g == 0), stop=(g == SH_NI // 128 - 1))
        nc.scalar.copy(out=pay3[:SIZE, 0:SIZE], in_=praw[:])
        nc.vector.reduce_sum(out=pay3[:SIZE, SIZE:SIZE + 1], in_=xiT[:], axis=AX.X)

        # ---------------- AG3 + combine
        ag3_in = dram.tile([128, AG3F], f32, name="ag3_in")
        nc.sync.dma_start(out=ag3_in[:], in_=pay3[:])
        nc.gpsimd.collective_compute(
            "AllGather", ALU.bypass, ins=[ag3_in[:].opt()],
            outs=[ag3_out[:].opt()], replica_groups=[list(range(NCORES))])
        ag3l = sb.tile([128, NCORES, AG3F], f32, name="ag3l")
        nc.sync.dma_start(out=ag3l[:],
                          in_=ag3_out[:].rearrange("(c p) f -> p c f", p=128))
        sum3 = sb.tile([128, NADD], f32, name="sum3")
        nc.vector.tensor_tensor(out=sum3[:], in0=ag3l[:, 0, 0:NADD],
                                in1=ag3l[:, 1, 0:NADD], op=ALU.add)
        for c in range(2, NCORES):
            nc.vector.tensor_tensor(out=sum3[:], in0=sum3[:],
                                    in1=ag3l[:, c, 0:NADD], op=ALU.add)
        dmin = sb.tile([128, 32], f32, name="dmin")
        nc.vector.tensor_tensor(out=dmin[:], in0=ag3l[:, 0, NADD:AG3F],
                                in1=ag3l[:, 1, NADD:AG3F], op=ALU.min)
        for c in range(2, NCORES):
            nc.vector.tensor_tensor(out=dmin[:], in0=dmin[:],
                                    in1=ag3l[:, c, NADD:AG3F], op=ALU.min)
        dmin16 = sb.tile([128, 16], f32, name="dmin16")
        dmv = dmin[:].rearrange("p (i h) -> p i h", h=2)
        nc.vector.tensor_tensor(out=dmin16[:], in0=dmv[:, :, 0], in1=dmv[:, :, 1],
                                op=ALU.min)
        dsum = sb.tile([128, 1], f32, name="dsum")
        nc.vector.reduce_sum(out=dsum[:], in_=dmin16[:], axis=AX.X)

        # ---------------- final assembly (fp32 [64,64])
        S64 = SIZE

        def new64(tag):
            return sb.tile([S64, S64], f32, tag=tag, name=tag)

        csum = sb.tile([S64, 1], f32, name="csum")
        nc.vector.tensor_copy(out=csum[:], in_=sum3[:S64, S64:S64 + 1])
        cr_ps = ps_sm.tile([1, S64], f32, tag="sm", name="cr_ps")
        nc.tensor.transpose(out=cr_ps[:], in_=csum[:], identity=ident_32[:S64, :S64])
        csr = sb.tile([1, S64], f32, name="csr")
        nc.scalar.copy(out=csr[:], in_=cr_ps[:])
        mr = sb.tile([1, S64], f32, name="mr")
        nc.scalar.activation(out=mr[:], in_=csr[:], func=AF.Copy, scale=1.0 / NIND)
        outer_ps = ps_sm.tile([S64, S64], f32, tag="sm", name="outer_ps")
        nc.tensor.matmul(out=outer_ps[:], lhsT=mr[:], rhs=csr[:],
                         start=True, stop=True)
        S_t = new64("S_t")
        nc.vector.tensor_tensor(out=S_t[:], in0=sum3[:S64, 0:S64], in1=outer_ps[:],
                                op=ALU.subtract)
        dtmp = new64("dtmp")
        nc.vector.tensor_tensor(out=dtmp[:], in0=S_t[:], in1=eye[:], op=ALU.mult)
        s2 = sb.tile([S64, 1], f32, name="s2")
        nc.vector.reduce_sum(out=s2[:], in_=dtmp[:], axis=AX.X)
        r2 = sb.tile([S64, 1], f32, name="r2")
        nc.vector.reciprocal(out=r2[:], in_=s2[:])
        s2r_ps = ps_sm.tile([1, S64], f32, tag="sm", name="s2r_ps")
        nc.tensor.transpose(out=s2r_ps[:], in_=s2[:], identity=ident_32[:S64, :S64])
        s2row = sb.tile([1, S64], f32, name="s2row")
        nc.scalar.copy(out=s2row[:], in_=s2r_ps[:])
        onesr64 = sb.tile([1, S64], f32, tag="onesr64", name="onesr64")
        nc.vector.memset(onesr64[:], 1.0)
        s2b_ps = ps_sm.tile([S64, S64], f32, tag="sm", name="s2b_ps")
        nc.tensor.matmul(out=s2b_ps[:], lhsT=onesr64[:], rhs=s2row[:],
                         start=True, stop=True)
        s2b = new64("s2b")
        nc.scalar.copy(out=s2b[:], in_=s2b_ps[:])
        SS = new64("SS")
        nc.vector.tensor_tensor(out=SS[:], in0=S_t[:], in1=S_t[:], op=ALU.mult)
        F_t = new64("F_t")
        nc.vector.tensor_scalar_mul(out=F_t[:], in0=SS[:], scalar1=r2[:])
        dg = new64("dg")
        nc.vector.tensor_tensor(out=dg[:], in0=s2b[:], in1=F_t[:], op=ALU.subtract)
        nc.vector.tensor_tensor(out=dg[:], in0=dg[:], in1=eye[:], op=ALU.add)
        B_t = new64("B_t")
        nc.vector.reciprocal(out=B_t[:], in_=dg[:])
        nc.vector.tensor_tensor(out=B_t[:], in0=B_t[:], in1=offd[:], op=ALU.mult)
        C_t = new64("C_t")
        nc.vector.tensor_tensor(out=C_t[:], in0=Lc[:], in1=LTc[:], op=ALU.subtract)
        nc.scalar.activation(out=C_t[:], in_=C_t[:], func=AF.Sigmoid)
        nc.vector.tensor_tensor(out=C_t[:], in0=C_t[:], in1=offd[:], op=ALU.mult)
        CT_t = new64("CT_t")
        nc.vector.tensor_tensor(out=CT_t[:], in0=LTc[:], in1=Lc[:], op=ALU.subtract)
        nc.scalar.activation(out=CT_t[:], in_=CT_t[:], func=AF.Sigmoid)
        nc.vector.tensor_tensor(out=CT_t[:], in0=CT_t[:], in1=offd[:], op=ALU.mult)
        U_t = new64("U_t")
        nc.vector.tensor_tensor(out=U_t[:], in0=CT_t[:], in1=C_t[:], op=ALU.add)
        cc_ps = ps_sm.tile([S64, S64], f32, tag="sm", name="cc_ps")
        nc.tensor.matmul(out=cc_ps[:], lhsT=CT_t[:], rhs=C_t[:],
                         start=True, stop=True)
        lt_t = new64("lt_t")
        nc.vector.tensor_tensor(out=lt_t[:], in0=cc_ps[:], in1=CT_t[:], op=ALU.mult)
        fin64 = sb.tile([S64, 8], f32, name="fin64")
        nc.vector.reduce_sum(out=fin64[:, 0:1], in_=lt_t[:], axis=AX.X)
        P_t = new64("P_t")
        nc.vector.tensor_tensor(out=P_t[:], in0=U_t[:], in1=B_t[:], op=ALU.mult)
        Q_t = new64("Q_t")
        nc.vector.tensor_tensor(out=Q_t[:], in0=C_t[:], in1=B_t[:], op=ALU.mult)
        ptq_ps = ps_sm.tile([S64, S64], f32, tag="sm", name="ptq_ps")
        nc.tensor.matmul(out=ptq_ps[:], lhsT=P_t[:], rhs=Q_t[:],
                         start=True, stop=True)
        t1_t = new64("t1_t")
        nc.vector.tensor_tensor(out=t1_t[:], in0=SS[:], in1=ptq_ps[:], op=ALU.mult)
        nc.vector.reduce_sum(out=fin64[:, 1:2], in_=t1_t[:], axis=AX.X)
        A_t = new64("A_t")
        nc.vector.tensor_tensor(out=A_t[:], in0=P_t[:], in1=S_t[:], op=ALU.mult)
        Bt_t = new64("Bt_t")
        nc.vector.tensor_tensor(out=Bt_t[:], in0=Q_t[:], in1=S_t[:], op=ALU.mult)
        nc.vector.tensor_scalar_mul(out=Bt_t[:], in0=Bt_t[:], scalar1=r2[:])
        ab_ps = ps_sm.tile([S64, S64], f32, tag="sm", name="ab_ps")
        nc.tensor.matmul(out=ab_ps[:], lhsT=A_t[:], rhs=Bt_t[:],
                         start=True, stop=True)
        t2_t = new64("t2_t")
        nc.vector.tensor_tensor(out=t2_t[:], in0=S_t[:], in1=ab_ps[:], op=ALU.mult)
        nc.vector.reduce_sum(out=fin64[:, 2:3], in_=t2_t[:], axis=AX.X)
        g1 = new64("t1_t")
        nc.vector.tensor_tensor(out=g1[:], in0=P_t[:], in1=SS[:], op=ALU.mult)
        gc = sb.tile([S64, 1], f32, tag="gcol", name="gcol")
        nc.vector.reduce_sum(out=gc[:], in_=g1[:], axis=AX.X)
        d1 = new64("t2_t")
        nc.vector.tensor_tensor(out=d1[:], in0=Q_t[:], in1=SS[:], op=ALU.mult)
        dc = sb.tile([S64, 1], f32, tag="dcol", name="dcol")
        nc.vector.reduce_sum(out=dc[:], in_=d1[:], axis=AX.X)
        t3c = sb.tile([S64, 1], f32, tag="t3col", name="t3col")
        nc.vector.tensor_tensor(out=t3c[:], in0=gc[:], in1=dc[:], op=ALU.mult)
        nc.vector.tensor_tensor(out=t3c[:], in0=t3c[:], in1=r2[:], op=ALU.mult)
        nc.vector.tensor_tensor(out=t3c[:], in0=t3c[:], in1=r2[:], op=ALU.mult)
        nc.vector.tensor_copy(out=fin64[:, 3:4], in_=t3c[:])
        t4_t = new64("lt_t")
        nc.vector.tensor_tensor(out=t4_t[:], in0=U_t[:], in1=C_t[:], op=ALU.mult)
        nc.vector.reduce_sum(out=fin64[:, 4:5], in_=t4_t[:], axis=AX.X)
        r2b = new64("dtmp")
        nc.vector.reciprocal(out=r2b[:], in_=s2b[:])
        ss_t = new64("t1_t")
        nc.vector.tensor_tensor(out=ss_t[:], in0=F_t[:], in1=r2b[:], op=ALU.mult)
        nc.vector.tensor_tensor(out=ss_t[:], in0=ss_t[:], in1=offd[:], op=ALU.mult)
        nc.vector.reduce_sum(out=fin64[:, 5:6], in_=ss_t[:], axis=AX.X)
        nc.vector.tensor_copy(out=fin64[:, 6:7], in_=sum3[:S64, 65:66])
        nc.vector.memset(fin64[:, 7:8], 0.0)

        f64_ps = ps_sm.tile([1, 8], f32, tag="sm", name="f64_ps")
        nc.tensor.matmul(out=f64_ps[:], lhsT=ones64[:], rhs=fin64[:],
                         start=True, stop=True)
        frow = sb.tile([1, 8], f32, name="frow")
        nc.scalar.copy(out=frow[:], in_=f64_ps[:])
        fin128 = sb.tile([128, 2], f32, name="fin128")
        nc.vector.tensor_copy(out=fin128[:, 0:1], in_=dsum[:])
        nc.vector.tensor_copy(out=fin128[:, 1:2], in_=zpsq_col[:])
        f128_ps = ps_sm.tile([1, 2], f32, tag="sm", name="f128_ps")
        nc.tensor.matmul(out=f128_ps[:], lhsT=ones128[:], rhs=fin128[:],
                         start=True, stop=True)
        grow = sb.tile([1, 2], f32, name="grow")
        nc.scalar.copy(out=grow[:], in_=f128_ps[:])

        acc = sb.tile([1, 1], f32, name="acc_sc")
        tmp = sb.tile([1, 1], f32, tag="tmp_sc", name="tmp_sc")
        nc.vector.tensor_copy(out=acc[:], in_=frow[:, 0:1])
        nc.scalar.activation(out=tmp[:], in_=frow[:, 6:7], func=AF.Copy,
                             scale=1.0 / (NS * SIZE))
        nc.vector.tensor_tensor(out=acc[:], in0=acc[:], in1=tmp[:], op=ALU.add)
        nc.scalar.activation(out=tmp[:], in_=grow[:, 0:1], func=AF.Copy,
                             scale=1.0 / (BTR * LAT))
        nc.vector.tensor_tensor(out=acc[:], in0=acc[:], in1=tmp[:], op=ALU.add)
        nc.scalar.activation(out=tmp[:], in_=grow[:, 1:2], func=AF.Copy,
                             scale=0.25 / (BTR * LAT))
        nc.vector.tensor_tensor(out=acc[:], in0=acc[:], in1=tmp[:], op=ALU.add)
        nc.vector.tensor_tensor(out=acc[:], in0=acc[:], in1=frow[:, 1:2],
                                op=ALU.add)
        nc.scalar.activation(out=tmp[:], in_=frow[:, 2:3], func=AF.Copy,
                             scale=-2.0)
        nc.vector.tensor_tensor(out=acc[:], in0=acc[:], in1=tmp[:], op=ALU.add)
        nc.vector.tensor_tensor(out=acc[:], in0=acc[:], in1=frow[:, 3:4],
                                op=ALU.add)
        nc.vector.tensor_tensor(out=acc[:], in0=acc[:], in1=frow[:, 4:5],
                                op=ALU.subtract)
        nc.scalar.activation(out=tmp[:], in_=frow[:, 5:6], func=AF.Copy,
                             scale=float(S64 - 2))
        nc.vector.tensor_tensor(out=acc[:], in0=acc[:], in1=tmp[:], op=ALU.add)
        nc.sync.dma_start(out=out_d[:], in_=acc[:])

    _split_multi_waits(nc)
    return nc


def _stage_inputs(I):
    g = lambda k: np.asarray(I[k], dtype=np.float32)
    z = g("z_logits")
    X = g("X")
    ntr = g("noise_trans")
    nind = g("noise_indep")
    perm = np.asarray(I["perm_idx"], dtype=np.int32).reshape(-1)
    L = g("conn_logits")

    def bf(a):
        return np.ascontiguousarray(a.astype(bfnp))

    def f(a):
        return np.ascontiguousarray(a.astype(np.float32))

    z_e32 = np.concatenate([z, np.ones((NS, 1), np.float32)], axis=1)
    shared = {
        "zext": bf(z_e32),
        "ntrT": bf(ntr.T),
        "ntr_ext": bf(np.concatenate([ntr, np.ones((BTR, 1), np.float32)], 1)),
        "nind_e32": f(np.concatenate([nind, np.ones((NIND, 1), np.float32)], 1)),
        "gW1T_bf": bf(g("glo_W1").T), "gW1nat_bf": bf(g("glo_W1")),
        "gW1T_32": f(g("glo_W1").T),
        "gW2T_bf": bf(g("glo_W2").T), "gW2T_32": f(g("glo_W2").T),
        "tW1T_bf": bf(g("tr_W1").T), "tW1nat_bf": bf(g("tr_W1")),
        "tW1T_32": f(g("tr_W1").T),
        "tW2T_bf": bf(g("tr_W2").T), "tW2T_32": f(g("tr_W2").T),
        "g_gam": f(g("glo_gamma").reshape(-1, 1)),
        "g_bet": f(g("glo_beta").reshape(-1, 1)),
        "t_gam": f(g("tr_gamma").reshape(-1, 1)),
        "t_bet": f(g("tr_beta").reshape(-1, 1)),
        "g_b2": f(g("glo_b2").reshape(-1, 1)),
        "t_b2": f(g("tr_b2").reshape(-1, 1)),
        "L32": f(L), "LT32": f(L.T),
        "eye64": np.eye(SIZE, dtype=np.float32),
        "offd64": (1.0 - np.eye(SIZE)).astype(np.float32),
        "ident_bf": np.eye(128, dtype=bfnp),
        "ident_32": np.eye(128, dtype=np.float32),
        "identr": np.eye(128, dtype=np.float32),
        "ones_row_bf": np.ones((1, 128), bfnp),
        "ones_col_bf": np.ones((128, 1), bfnp),
        "ones64_32": np.ones((SIZE, 1), np.float32),
        "ones128_32": np.ones((128, 1), np.float32),
    }
    zT = z.T
    XT = X.T
    nindT = nind.T
    maps = []
    for c in range(NCORES):
        m = dict(shared)
        m["znat32"] = f(z_e32[c * SH_NS:(c + 1) * SH_NS, :])
        m["zT_sh"] = bf(zT[:, c * SH_NS:(c + 1) * SH_NS])
        m["xT_sh"] = bf(XT[:, c * SH_NS:(c + 1) * SH_NS])
        m["nindT32"] = f(nindT[:, c * SH_NI:(c + 1) * SH_NI])
        m["perm_sh"] = np.ascontiguousarray(
            perm[c * SH_J:(c + 1) * SH_J].reshape(NI_CH, 128).T)
        maps.append(m)
    return maps


def _get_nc():
    if "nc" not in _CACHE:
        _install_profshim()
        _CACHE["nc"] = _build_program()
    return _CACHE["nc"]


def run(inputs, trace=False):
    nc = _get_nc()
    maps = _stage_inputs(inputs)
    res = run_bass_kernel_spmd(nc, maps, list(range(NCORES)), trace=trace)
    val = np.float32(res.results[0]["out"].reshape(-1)[0])
    return val, res


def kernel(**inputs) -> np.ndarray:
    val, _ = run(inputs, trace=False)
    return np.asarray(val, dtype=np.float32)


if __name__ == "__main__":
    nc = _get_nc()
    ninst = sum(len(bb.instructions) for bb in nc.main_func.blocks)
    print("built ok, instructions:", ninst)
